# revision 15
# baseline (speedup 1.0000x reference)
"""Trainium2 Bass kernel for a 2-layer edge-conditioned GAT (PyG GATConv style).

Strategy (8 NeuronCores, SPMD, node-parallel):
  - Nodes dealt to cores round-robin in degree order; each core owns softmax +
    aggregation for its nodes.  Per core, nodes are bucketed into 128-lane
    chunks; incoming edges form a padded [lane, slot] grid so per-edge ops are
    dense tile ops.
  - The per-layer gather table ([xl | 1 | a_src] per node, 66 fp16 cols in
    256B-strided rows) is computed REPLICATED on every core by a cheap dense
    matmul pass (layer 1 from the replicated x; layer 2 from an fp16
    AllGather of h^T).  No table AllGather.
  - Edge gathers use SWDGE dma_gather with int16 indices.  The >32768-row
    range is covered by NW=5 overlapping 32768-row windows; edges in window
    overlaps are assigned to windows so as to minimize the padded per-chunk
    grid widths (min-cost interval assignment, SPMD-uniform across cores).
  - Scores: e = lrelu(a_src[src] + a_dst[dst] + c*ea); the max-shift of the
    reference softmax is replaced by a constant shift (exact: softmax is
    shift invariant; scores are bounded).  The appended all-ones table column
    makes the softmax denominator fall out of the same fused multiply+reduce
    that aggregates features.
"""

import math

import numpy as np

NCORE = 8
ROW = 128          # fp16 elements per gather-table row (= 256B, SWDGE minimum)
TCOL = 66          # used table columns: [xl(0:64) | one | a_src]
B_MAX = 150        # max gather blocks (of 128 edges) per group
SUB_BLK = 8        # blocks per dma_gather call (1024 idxs = Q7 ucode scratch cap)
SCRATCH = 32768    # SWDGE ring carveout (2048 descriptors = 2 calls in flight)
                   # = 2 gather calls in flight: prep overlaps transfer)
LIM = 32768        # int16 gather window (rows per window)
NW = 5             # gather windows
EXP_SHIFT = -8.0   # constant softmax shift
PAD_AE = -60000.0  # score for padded slots -> exp == 0

_BUILD_CACHE = {}


# ----------------------------------------------------------------------------
# Host-side preprocessing
# ----------------------------------------------------------------------------

def _pack(src_p, dst_core, dst_chunk, dst_lane, NCHUNK, S):
    """4-window grid packing.  Returns per-chunk per-window widths W [NCHUNK,NW],
    and per-edge (window, depth) assignments.  Widths are shared across cores
    (SPMD-uniform program)."""
    EE = len(src_p)
    # window-coverage interval [lo, hi] per edge (coverage is contiguous)
    lo = np.zeros(EE, np.int8)
    hi = np.zeros(EE, np.int8)
    cov = np.zeros((NW, EE), bool)
    for w in range(NW):
        cov[w] = (src_p >= S[w]) & (src_p < S[w] + LIM)
    lo = np.argmax(cov, axis=0).astype(np.int8)
    hi = (NW - 1 - np.argmax(cov[::-1], axis=0)).astype(np.int8)
    assert (cov[lo, np.arange(EE)] & cov[hi, np.arange(EE)]).all()

    gkey = (dst_core.astype(np.int64) * NCHUNK + dst_chunk) * 128 + dst_lane
    NKEY = NCORE * NCHUNK * 128

    # per-key demand per class (class = (lo, hi) pair); classes are few
    classes = sorted({(int(a), int(b)) for a, b in zip(lo, hi)})
    cidx = {c: i for i, c in enumerate(classes)}
    ecls = np.array([cidx[(int(a), int(b))] for a, b in zip(lo, hi)],
                    dtype=np.int8)
    NCLS = len(classes)
    dem_cls = np.zeros((NKEY, NCLS), np.int32)
    np.add.at(dem_cls, (gkey, ecls), 1)

    key_chunk = (np.arange(NKEY) // 128) % NCHUNK

    # interval-constraint DP for per-chunk widths (joint across cores):
    # c[k+1] = max_i (c[i] + dem[i,k]) where dem[i,k] = max over keys of
    # edges whose interval is within [i, k]
    W = np.zeros((NCHUNK, NW), np.int64)
    for j in range(NCHUNK):
        sel = key_chunk == j
        dj = dem_cls[sel]
        dem = {}
        for i in range(NW):
            for k in range(i, NW):
                csel = [cidx[c] for c in classes
                        if c[0] >= i and c[1] <= k]
                dem[(i, k)] = int(dj[:, csel].sum(axis=1).max()) if csel else 0
        c = [0] * (NW + 1)
        for k in range(NW):
            c[k + 1] = max([c[i] + dem[(i, k)] for i in range(k + 1)] + [c[k]])
        c[1] = max(c[1], 1)
        for k in range(1, NW):
            c[k + 1] = max(c[k + 1], c[k])
        W[j] = np.diff(np.array(c))
    assert (W[:, 0] >= 1).all()

    # per-key greedy assignment: for w in 0..NW-1 take classes by ascending hi
    remaining = dem_cls.astype(np.int64).copy()
    take = np.zeros((NKEY, NCLS, NW), np.int32)   # edges of class -> window
    order = sorted(range(NCLS), key=lambda ci: (classes[ci][1], classes[ci][0]))
    for w in range(NW):
        cap = W[key_chunk, w].copy()
        for ci in order:
            clo, chi = classes[ci]
            if not (clo <= w <= chi):
                continue
            if chi == w:
                t = remaining[:, ci].copy()   # must take all
            else:
                t = np.minimum(remaining[:, ci], cap)
            take[:, ci, w] = t
            cap -= t
            remaining[:, ci] -= t
        assert (cap >= 0).all(), f"window {w} overflow"
    assert (remaining == 0).all(), "assignment infeasible"

    # per-edge window: position within (key, class) decides the window
    eorder = np.lexsort((ecls, gkey))
    kc_sorted = gkey[eorder] * np.int64(NCLS) + ecls[eorder]
    starts = np.r_[0, np.flatnonzero(np.diff(kc_sorted)) + 1]
    counts = np.diff(np.r_[starts, EE])
    posin = np.arange(EE) - np.repeat(starts, counts)
    cum = np.cumsum(take, axis=2)               # [NKEY, NCLS, NW]
    pos_e = np.empty(EE, np.int64)
    pos_e[eorder] = posin
    cum_e = cum[gkey, ecls]                     # [EE, NW]
    win_e = (pos_e[:, None] >= cum_e).sum(axis=1).astype(np.int8)
    assert (win_e < NW).all()
    assert (cov[win_e, np.arange(EE)]).all()

    # depth within (key, window)
    kw = gkey * np.int64(NW) + win_e
    o2 = np.argsort(kw, kind="stable")
    kw_s = kw[o2]
    st2 = np.r_[0, np.flatnonzero(np.diff(kw_s)) + 1]
    cn2 = np.diff(np.r_[st2, EE])
    k_s = np.arange(EE) - np.repeat(st2, cn2)
    depth_e = np.empty(EE, np.int64)
    depth_e[o2] = k_s
    assert (depth_e < W[dst_chunk, win_e]).all()
    return W, win_e, depth_e


def _prepare(x, edge_index, edge_attr,
             W_res, b_res, alpha_mix,
             W1, att_src1, att_dst1, We1, att_e1, b1,
             W2, att_src2, att_dst2, We2, att_e2, b2):
    N, D = x.shape
    E = edge_index.shape[1]
    f32 = np.float32

    src = np.concatenate([edge_index[0], np.arange(N, dtype=np.int64)]).astype(np.int64)
    dst = np.concatenate([edge_index[1], np.arange(N, dtype=np.int64)]).astype(np.int64)
    ea = np.concatenate([edge_attr[:, 0].astype(f32),
                         np.full(N, edge_attr.astype(f32).mean(), dtype=f32)])
    EE = E + N

    deg = np.bincount(dst, minlength=N)

    PCORE = int(math.ceil(N / NCORE / 128) * 128)
    NCHUNK = PCORE // 128
    TROWS = NCORE * PCORE
    B0 = TROWS - LIM
    S = tuple(int(round(i * B0 / (NW - 1))) for i in range(NW))

    # Deal nodes to cores round-robin in degree order (balances edge counts);
    # within a core sort by degree so chunk members have similar degree.
    order = np.argsort(deg, kind="stable")
    rank = np.empty(N, dtype=np.int64)
    rank[order] = np.arange(N)
    core_of = (rank % NCORE).astype(np.int32)
    pos_in_core = np.full(N, -1, dtype=np.int64)
    perm = np.full((NCORE, PCORE), -1, dtype=np.int64)
    for c in range(NCORE):
        nodes = np.where(core_of == c)[0]
        nodes = nodes[np.argsort(deg[nodes], kind="stable")]
        pos_in_core[nodes] = np.arange(len(nodes))
        perm[c, :len(nodes)] = nodes
    p_id = core_of.astype(np.int64) * PCORE + pos_in_core

    src_p = p_id[src]
    dst_core = core_of[dst]
    dst_chunk = pos_in_core[dst] // 128
    dst_lane = pos_in_core[dst] % 128

    W, win_e, depth_e = _pack(src_p, dst_core, dst_chunk, dst_lane, NCHUNK, S)

    # Greedy group packing: chunks -> groups with <= B_MAX blocks each.
    # Group packing with a ramp: small first groups so the vector engine
    # starts while later gathers stream, and a small final group so the
    # exposed tail compute is short.
    wsum = W.sum(axis=1)
    groups = []
    gs, acc = 0, 0
    for j in range(NCHUNK):
        b = int(wsum[j])
        cap = (B_MAX // 3 if len(groups) == 0 else
               2 * B_MAX // 3 if len(groups) == 1 else B_MAX)
        if acc + b > cap and j > gs:
            groups.append((gs, j))
            gs, acc = j, 0
        acc += b
    groups.append((gs, NCHUNK))
    # split an oversized tail group in half
    if len(groups) >= 2:
        a, b = groups[-1]
        if b - a >= 4 and wsum[a:b].sum() > B_MAX // 2:
            mid = a + (b - a) // 2
            groups[-1:] = [(a, mid), (mid, b)]

    # Block layout per group: [win0 blocks of its chunks | win1 | win2 | win3]
    base = np.zeros((NCHUNK, NW), np.int64)
    group_info = []   # (blk0, (nblk per window), (ca, cb))
    bpos = 0
    for (a, b) in groups:
        blk0 = bpos
        nblk = []
        for w in range(NW):
            n0 = bpos
            for j in range(a, b):
                base[j, w] = bpos
                bpos += int(W[j, w])
            nblk.append(bpos - n0)
        group_info.append((blk0, tuple(nblk), (a, b)))
    B_TOT = bpos
    NSLOT = B_TOT * 128

    blk_e = base[dst_chunk, win_e] + depth_e
    slot_e = blk_e * 128 + dst_lane
    idx_val = (src_p - np.asarray(S, dtype=np.int64)[win_e])
    assert (idx_val >= 0).all() and (idx_val < LIM).all()
    idx_val = idx_val.astype(np.int16)

    c1 = float(np.dot(We1[0].astype(f32), att_e1.astype(f32)))
    c2 = float(np.dot(We2[0].astype(f32), att_e2.astype(f32)))

    idx_imgs, ae1_imgs, ae2_imgs = [], [], []
    for c in range(NCORE):
        m = dst_core == c
        sl = slot_e[m]
        grid_idx = np.zeros(NSLOT, dtype=np.int16)
        grid_idx[sl] = idx_val[m]
        g1 = np.full(NSLOT, PAD_AE, dtype=f32)
        g1[sl] = c1 * ea[m]
        g2 = np.full(NSLOT, PAD_AE, dtype=f32)
        g2[sl] = c2 * ea[m]
        img16 = grid_idx.reshape(-1, 16).T
        # dma_gather on queue 0 reads 32 idx channels; 2x-replicated 16-wrap
        idx_imgs.append(np.tile(img16, (2, 1)).copy())
        ae1_imgs.append(np.ascontiguousarray(g1.reshape(B_TOT, 128).T.astype(np.float16)))
        ae2_imgs.append(np.ascontiguousarray(g2.reshape(B_TOT, 128).T.astype(np.float16)))

    # Give fully-padded lanes (node-count padding) one live slot so s > 0.
    lane_has = np.zeros((NCORE, PCORE), dtype=bool)
    lane_has[dst_core, pos_in_core[dst]] = True
    for c in range(NCORE):
        for j in range(NCHUNK):
            dead = np.where(~lane_has[c, j * 128:(j + 1) * 128])[0]
            if len(dead):
                ae1_imgs[c][dead, base[j, 0]] = 0.0
                ae2_imgs[c][dead, base[j, 0]] = 0.0

    # Weights.  Residual Linear folded into layer 1; biases folded via the
    # ones row of the lhsT (layer 1) / zero-bias (layer 2).
    W_res = W_res.astype(f32)
    b_res = b_res.astype(f32)
    W1 = W1.astype(f32)
    W2 = W2.astype(f32)
    alpha = float(alpha_mix)
    W1e = W_res @ W1
    b1e = b_res @ W1

    # Table pass rhs, layer 1: cols [xl(0:64) | one | a_src], lhsT rows [x; 1]
    Wb1_tab = np.zeros((D + 1, TCOL), dtype=f32)
    Wb1_tab[:D, 0:D] = W1e
    Wb1_tab[D, 0:D] = b1e
    Wb1_tab[D, D] = 1.0
    Wb1_tab[:D, D + 1] = W1e @ att_src1.astype(f32)
    Wb1_tab[D, D + 1] = float(b1e @ att_src1.astype(f32))
    # Own pass rhs, layer 1: cols [a_dst | alpha*xres(0:64)]
    Wb1_own = np.zeros((D + 1, 1 + D), dtype=f32)
    Wb1_own[:D, 0] = W1e @ att_dst1.astype(f32)
    Wb1_own[D, 0] = float(b1e @ att_dst1.astype(f32))
    Wb1_own[:D, 1:] = W_res * alpha
    Wb1_own[D, 1:] = b_res * alpha
    # Table pass rhs, layer 2 (lhsT rows [h; 1])
    Wb2_tab = np.zeros((D + 1, TCOL), dtype=f32)
    Wb2_tab[:D, 0:D] = W2
    Wb2_tab[D, D] = 1.0
    Wb2_tab[:D, D + 1] = W2 @ att_src2.astype(f32)
    # Own pass rhs, layer 2: col [a_dst2]
    Wb2_own = np.zeros((D + 1, 1), dtype=f32)
    Wb2_own[:D, 0] = W2 @ att_dst2.astype(f32)

    # Dense inputs: x^T in p_id order with ones row, fp16.
    xT_full = np.zeros((D + 1, TROWS), dtype=np.float16)
    valid_all = perm.reshape(-1) >= 0
    cols = np.arange(TROWS)[valid_all]
    xT_full[:D, cols] = x[perm.reshape(-1)[valid_all]].astype(np.float16).T
    xT_full[D, :] = 1.0

    WMAXW = int(W.max())
    WMAXC = int(W.sum(axis=1).max())

    cfg = dict(
        N=N, D=D, PCORE=PCORE, NCHUNK=NCHUNK, TROWS=TROWS,
        B_TOT=B_TOT, NSLOT=NSLOT, WMAXW=WMAXW, WMAXC=WMAXC, S=S,
        W=tuple(tuple(int(v) for v in row) for row in W),
        base=tuple(tuple(int(v) for v in row) for row in base),
        groups=tuple((int(b0), tuple(int(n) for n in nblk), (int(a), int(b)))
                     for (b0, nblk, (a, b)) in group_info),
    )

    in_maps = []
    for c in range(NCORE):
        in_maps.append(dict(
            xT_full=xT_full,
            xT_own=np.ascontiguousarray(xT_full[:, c * PCORE:(c + 1) * PCORE]),
            idx_img=idx_imgs[c],
            ae1=ae1_imgs[c],
            ae2=ae2_imgs[c],
            Wb1_tab=Wb1_tab.astype(np.float16),
            Wb1_own=Wb1_own.astype(np.float16),
            Wb2_tab=Wb2_tab.astype(np.float16),
            Wb2_own=Wb2_own.astype(np.float16),
            b1row=np.tile(b1.astype(f32).reshape(1, D), (128, 1)),
            b2row=np.tile(b2.astype(f32).reshape(1, D), (128, 1)),
            ones_row=np.ones((1, PCORE), dtype=np.float16),
            ident=np.eye(128, dtype=np.float16),
        ))
    return cfg, in_maps, perm


# ----------------------------------------------------------------------------
# Device program
# ----------------------------------------------------------------------------

def _build(cfg_key, stage='full'):
    import contextlib

    import concourse.bass as bass
    import concourse.tile as tile
    import concourse.mybir as mybir
    from concourse import bacc
    from concourse.library_config import mlp

    cfg = dict(cfg_key)
    D = cfg["D"]
    PCORE, NCHUNK, TROWS = cfg["PCORE"], cfg["NCHUNK"], cfg["TROWS"]
    B_TOT, NSLOT, WMAXW = cfg["B_TOT"], cfg["NSLOT"], cfg["WMAXW"]
    WMAXC = cfg["WMAXC"]
    S = cfg["S"]
    W = cfg["W"]
    base = cfg["base"]
    groups = cfg["groups"]

    fp16 = mybir.dt.float16
    fp32 = mybir.dt.float32
    i16 = mybir.dt.int16
    AF = mybir.ActivationFunctionType
    ALU = mybir.AluOpType

    nc = bacc.Bacc("TRN2", target_bir_lowering=False, debug=False,
                   num_devices=NCORE, dynamic_dma_scratch_size=SCRATCH)

    xT_full_d = nc.dram_tensor("xT_full", [D + 1, TROWS], fp16, kind="ExternalInput")
    xT_own_d = nc.dram_tensor("xT_own", [D + 1, PCORE], fp16, kind="ExternalInput")
    idx_img = nc.dram_tensor("idx_img", [32, NSLOT // 16], i16, kind="ExternalInput")
    ae1_d = nc.dram_tensor("ae1", [128, B_TOT], fp16, kind="ExternalInput")
    ae2_d = nc.dram_tensor("ae2", [128, B_TOT], fp16, kind="ExternalInput")
    Wb1_tab_d = nc.dram_tensor("Wb1_tab", [D + 1, TCOL], fp16, kind="ExternalInput")
    Wb1_own_d = nc.dram_tensor("Wb1_own", [D + 1, 1 + D], fp16, kind="ExternalInput")
    Wb2_tab_d = nc.dram_tensor("Wb2_tab", [D + 1, TCOL], fp16, kind="ExternalInput")
    Wb2_own_d = nc.dram_tensor("Wb2_own", [D + 1, 1], fp16, kind="ExternalInput")
    b1row_d = nc.dram_tensor("b1row", [128, D], fp32, kind="ExternalInput")
    b2row_d = nc.dram_tensor("b2row", [128, D], fp32, kind="ExternalInput")
    ones_d = nc.dram_tensor("ones_row", [1, PCORE], fp16, kind="ExternalInput")
    ident_d = nc.dram_tensor("ident", [128, 128], fp16, kind="ExternalInput")
    y_d = nc.dram_tensor("y", [PCORE, D], fp32, kind="ExternalOutput")

    T_d = [nc.dram_tensor(f"T{l}", [TROWS, ROW], fp16) for l in range(2)]

    # Piece split of the group list: the h AllGather is pipelined in NQ
    # column pieces, each issued as soon as its chunks' epilogue is done so
    # the collective overlaps the remaining edge-phase groups.
    NGRP = len(groups)
    # Skewed boundaries: the last AllGather piece is the only one whose
    # latency is exposed (it starts at edge-phase-1's end), so make it the
    # final group alone (small); split the rest evenly.
    NQ = min(6, NGRP)
    qbound = [ (q + 1) * (NGRP - 1) // (NQ - 1) for q in range(NQ - 1) ]
    qbound.append(NGRP)  # end-group (exclusive) per piece
    qruns = []
    g0 = 0
    for q in range(NQ):
        g1 = qbound[q]
        ca = groups[g0][2][0]
        cb = groups[g1 - 1][2][1]
        qruns.append((g1 - 1, ca, cb))
        g0 = g1
    h_tin_q = [nc.dram_tensor(f"h_tin{q}", [D, (cb - ca) * 128], fp16)
               for q, (_, ca, cb) in enumerate(qruns)]
    h_tall_q = [nc.dram_tensor(f"h_tall{q}", [NCORE * D, (cb - ca) * 128], fp16,
                               addr_space="Shared")
                for q, (_, ca, cb) in enumerate(qruns)]

    nc.gpsimd.load_library(mlp)
    rg = [list(range(NCORE))]

    with tile.TileContext(nc) as tc:
        with contextlib.ExitStack() as ctx:
            resident = ctx.enter_context(tc.tile_pool(name="resident", bufs=1))
            slab = ctx.enter_context(tc.tile_pool(name="slab", bufs=2))
            gpool = ctx.enter_context(tc.tile_pool(name="gather", bufs=2))
            ppool = ctx.enter_context(tc.tile_pool(name="prod", bufs=2))
            spool = ctx.enter_context(tc.tile_pool(name="small", bufs=3))
            epool = ctx.enter_context(tc.tile_pool(name="epil", bufs=2))
            dpool = ctx.enter_context(tc.tile_pool(name="dense", bufs=2))
            ipool = ctx.enter_context(tc.tile_pool(name="idx", bufs=2))
            psum_p = ctx.enter_context(tc.tile_pool(name="ps", bufs=6, space="PSUM"))
            psum_t = ctx.enter_context(tc.tile_pool(name="pst", bufs=2, space="PSUM"))

            # ---------------- resident loads ----------------
            Wb1_tab_sb = resident.tile([D + 1, TCOL], fp16)
            nc.sync.dma_start(Wb1_tab_sb[:], Wb1_tab_d.ap())
            Wb1_own_sb = resident.tile([D + 1, 1 + D], fp16)
            nc.sync.dma_start(Wb1_own_sb[:], Wb1_own_d.ap())
            Wb2_tab_sb = resident.tile([D + 1, TCOL], fp16)
            nc.sync.dma_start(Wb2_tab_sb[:], Wb2_tab_d.ap())
            Wb2_own_sb = resident.tile([D + 1, 1], fp16)
            nc.sync.dma_start(Wb2_own_sb[:], Wb2_own_d.ap())
            b1row = resident.tile([128, D], fp32)
            nc.sync.dma_start(b1row[:], b1row_d.ap())
            b2row = resident.tile([128, D], fp32)
            nc.sync.dma_start(b2row[:], b2row_d.ap())
            ident = resident.tile([128, 128], fp16)
            nc.sync.dma_start(ident[:], ident_d.ap())
            expshift = resident.tile([128, 1], fp32)
            nc.vector.memset(expshift[:], EXP_SHIFT)

            h_T = resident.tile([D + 1, PCORE], fp16)
            nc.sync.dma_start(h_T[D:D + 1, :], ones_d.ap())

            ae_sb = [resident.tile([128, B_TOT], fp16, name=f"ae_sb{l}")
                     for l in range(2)]
            aeadst = [resident.tile([128, B_TOT], fp16, name=f"aeadst{l}")
                      for l in range(2)]
            xres16 = resident.tile([128, NCHUNK * D], fp16)
            h_sb = resident.tile([128, NCHUNK * D], fp16)
            adst = [resident.tile([128, NCHUNK], fp32, name=f"adst{l}")
                    for l in range(2)]
            pre_buf = resident.tile([128, NCHUNK * D], fp32)

            def table_pass(layer, lhsT_src):
                """Write the full gather table T[layer] from dense matmuls.
                lhsT_src(s) -> loads slab s ([D+1, PCORE]) and returns tile.
                One batched DMA write per slab (per-chunk writes serialize on
                the HWDGE fixed overhead); PSUM->SBUF staging alternates
                between the Activation and Vector engines."""
                half = (NCHUNK + 2) // 3
                for s in range(NCORE):
                    xs = lhsT_src(s)
                    for j0 in range(0, NCHUNK, half):
                        j1 = min(j0 + half, NCHUNK)
                        tb = dpool.tile([128, half, TCOL], fp16, tag="tabs")
                        for j in range(j0, j1):
                            ps = psum_p.tile([128, TCOL], fp32, tag="dps")
                            nc.tensor.matmul(ps[:], xs[:, j * 128:(j + 1) * 128],
                                             (Wb1_tab_sb if layer == 0 else Wb2_tab_sb)[:],
                                             start=True, stop=True)
                            if j % 2 == 0:
                                nc.scalar.activation(tb[:, j - j0, :], ps[:], AF.Copy)
                            else:
                                nc.vector.tensor_copy(tb[:, j - j0, :], ps[:])
                        r0 = (s * NCHUNK + j0) * 128
                        nc.sync.dma_start(
                            T_d[layer].ap()[r0:r0 + (j1 - j0) * 128, 0:TCOL]
                            .rearrange("(b l) c -> l b c", l=128),
                            tb[:, 0:j1 - j0, :])

            def own_pass(layer, ja=0, jb=NCHUNK, lhsT=None):
                if layer == 0:
                    xo = slab.tile([D + 1, PCORE], fp16, tag="slab")
                    nc.sync.dma_start(xo[:], xT_own_d.ap())
                    lhsT = xo
                elif lhsT is None:
                    lhsT = h_T
                ncols = (1 + D) if layer == 0 else 1
                W_own = Wb1_own_sb if layer == 0 else Wb2_own_sb
                for j in range(ja, jb):
                    ps = psum_p.tile([128, TCOL], fp32, tag="dps")
                    nc.tensor.matmul(ps[:, 0:ncols], lhsT[:, j * 128:(j + 1) * 128],
                                     W_own[:], start=True, stop=True)
                    nc.vector.tensor_copy(adst[layer][:, j:j + 1], ps[:, 0:1])
                    if layer == 0:
                        nc.scalar.activation(
                            xres16[:, j * D:(j + 1) * D], ps[:, 1:1 + D], AF.Copy)

            def quarter_epilogue(q):
                """h = elu(pre + b1) for quarter q's chunks, transpose into
                h_T, write h_tin[q] and kick its AllGather.  Emitted mid
                edge-phase-1 so the collective overlaps later groups."""
                _, ca, cb = qruns[q]
                for j0 in range(ca, cb, 4):
                    j1 = min(j0 + 4, cb)
                    b0, b1_ = j0 * D, j1 * D
                    w = b1_ - b0
                    nj = j1 - j0
                    t0 = epool.tile([128, 4 * D], fp32, tag="eb0")
                    nc.vector.tensor_tensor(
                        t0[:, 0:w].rearrange("l (j c) -> l j c", c=D),
                        pre_buf[:, b0:b1_].rearrange("l (j c) -> l j c", c=D),
                        b1row[:].unsqueeze(1).broadcast_to([128, nj, D]), ALU.add)
                    mneg = epool.tile([128, 4 * D], fp32, tag="eb1")
                    nc.vector.tensor_scalar_min(mneg[:, 0:w], t0[:, 0:w], 0.0)
                    eneg = epool.tile([128, 4 * D], fp32, tag="eb2")
                    nc.scalar.activation(eneg[:, 0:w], mneg[:, 0:w], AF.Exp)
                    ppos = epool.tile([128, 4 * D], fp32, tag="eb1b")
                    nc.vector.tensor_scalar_max(ppos[:, 0:w], t0[:, 0:w], 0.0)
                    nc.vector.scalar_tensor_tensor(
                        h_sb[:, b0:b1_], eneg[:, 0:w], -1.0, ppos[:, 0:w],
                        ALU.add, ALU.add)
                for j in range(ca, cb):
                    pt = psum_t.tile([D, 128], fp16, tag="pt")
                    nc.tensor.transpose(pt[:], h_sb[:, j * D:(j + 1) * D], ident[:])
                    nc.vector.tensor_copy(h_T[0:D, j * 128:(j + 1) * 128], pt[:])
                # layer-2 dense prep for these chunks (h_T cols just written)
                own_pass(1, ca, cb)
                prep_aeadst(1, ca, cb)

            def launch_collective(q):
                """h_tin write + AllGather for piece q.  Emitted one group
                after the epilogue compute so its sem waits don't stall the
                in-order SP/Pool queues mid-stream."""
                _, ca, cb = qruns[q]
                nc.sync.dma_start(h_tin_q[q].ap(),
                                  h_T[0:D, ca * 128:cb * 128])
                nc.gpsimd.collective_compute(
                    "AllGather", ALU.bypass, replica_groups=rg,
                    ins=[h_tin_q[q].ap().opt()], outs=[h_tall_q[q].ap().opt()])

            def load_ae(layer):
                nc.sync.dma_start(ae_sb[layer][:],
                                  (ae1_d if layer == 0 else ae2_d).ap())

            def prep_aeadst(layer, ja=0, jb=NCHUNK):
                """ae + a_dst per slot for chunks [ja, jb)."""
                for j in range(ja, jb):
                    for w in range(NW):
                        if W[j][w]:
                            b0 = base[j][w]
                            nc.vector.tensor_scalar_add(
                                aeadst[layer][:, b0:b0 + W[j][w]],
                                ae_sb[layer][:, b0:b0 + W[j][w]],
                                adst[layer][:, j:j + 1])

            def edge_phase(layer, tasks=None):
                table = T_d[layer]
                for gi, (blk0, nblk, (ca, cb)) in enumerate(groups):
                    bg = sum(nblk)
                    G = gpool.tile([128, B_MAX, ROW], fp16, tag="G")
                    it = ipool.tile([32, B_MAX * 8], i16, tag="it")
                    nc.sync.dma_start(it[:, 0:bg * 8],
                                      idx_img.ap()[:, blk0 * 8:(blk0 + bg) * 8])
                    off = 0
                    for w in range(NW):
                        for s0 in range(0, nblk[w], SUB_BLK):
                            nb = min(SUB_BLK, nblk[w] - s0)
                            o = off + s0
                            nc.gpsimd.dma_gather(
                                G[:, o:o + nb, :],
                                table.ap()[S[w]:S[w] + LIM, :],
                                it[:, o * 8:(o + nb) * 8], nb * 128, nb * 128, ROW)
                        off += nblk[w]
                    u = spool.tile([128, B_MAX], fp32, tag="u")
                    nc.vector.tensor_tensor(
                        u[:, 0:bg], G[:, 0:bg, D + 1:D + 2].squeeze(2),
                        aeadst[layer][:, blk0:blk0 + bg], ALU.add)
                    t = spool.tile([128, B_MAX], fp32, tag="t")
                    nc.vector.scalar_tensor_tensor(
                        t[:, 0:bg], u[:, 0:bg], 0.2, u[:, 0:bg],
                        ALU.mult, ALU.max)
                    ex = spool.tile([128, B_MAX], fp16, tag="ex")
                    nc.scalar.activation(ex[:, 0:bg], t[:, 0:bg], AF.Exp,
                                         bias=expshift[:])
                    for j in range(ca, cb):
                        # P holds the chunk's windows back to back so one
                        # reduce covers the whole neighborhood.
                        P = ppool.tile([128, WMAXC, D + 1], fp16, tag="P")
                        po = 0
                        for w in range(NW):
                            dd = W[j][w]
                            if not dd:
                                continue
                            r0 = base[j][w] - blk0
                            nc.vector.tensor_tensor(
                                P[:, po:po + dd, :], G[:, r0:r0 + dd, 0:D + 1],
                                ex[:, r0:r0 + dd].unsqueeze(2)
                                .broadcast_to([128, dd, D + 1]),
                                ALU.mult)
                            po += dd
                        acc = spool.tile([128, D + 1], fp32, tag="red")
                        nc.vector.tensor_reduce(
                            acc[:], P[:, 0:po, :].transpose([0, 2, 1]),
                            axis=mybir.AxisListType.X, op=ALU.add)
                        rs = spool.tile([128, 1], fp32, tag="rs")
                        nc.vector.reciprocal(rs[:], acc[:, D:D + 1])
                        nc.vector.tensor_scalar_mul(
                            pre_buf[:, j * D:(j + 1) * D], acc[:, 0:D], rs[:])
                    if tasks:
                        for fn in tasks.get(gi, ()):
                            fn()

            def y_quarter(q):
                """y = pre + b2 + alpha*x_res for quarter q's chunks, written
                out as soon as they are reduced (overlaps later edge-2 groups)."""
                _, ca, cb = qruns[q]
                for j0 in range(ca, cb, 4):
                    j1 = min(j0 + 4, cb)
                    b0, b1_ = j0 * D, j1 * D
                    w = b1_ - b0
                    nj = j1 - j0
                    y0 = epool.tile([128, 4 * D], fp32, tag="eb0")
                    nc.vector.tensor_tensor(
                        y0[:, 0:w].rearrange("l (j c) -> l j c", c=D),
                        pre_buf[:, b0:b1_].rearrange("l (j c) -> l j c", c=D),
                        b2row[:].unsqueeze(1).broadcast_to([128, nj, D]),
                        ALU.add)
                    y1 = epool.tile([128, 4 * D], fp32, tag="eb1")
                    nc.vector.tensor_tensor(y1[:, 0:w], y0[:, 0:w],
                                            xres16[:, b0:b1_], ALU.add)
                    nc.sync.dma_start(
                        y_d.ap().rearrange("(j l) c -> l j c", l=128)
                        [:, j0:j1, :],
                        y1[:, 0:w].rearrange("l (j c) -> l j c", c=D))

            def finish_early():
                y_stub = spool.tile([128, D], fp32, tag="ystub")
                nc.vector.memset(y_stub[:], 0.0)
                nc.sync.dma_start(y_d.ap()[0:128, :], y_stub[:])

            WFREE = NCHUNK * D
            NB = 512

            # ================= layer 1 =================
            def x_slab(s):
                xs = slab.tile([D + 1, PCORE], fp16, tag="slab")
                nc.sync.dma_start(xs[:], xT_full_d.ap()[:, s * PCORE:(s + 1) * PCORE])
                return xs

            QMAX = max(cb - ca for (_, ca, cb) in qruns)

            def table2_build(q, s0, s1):
                """Table-2 rows for piece q, source slabs [s0, s1).  Paced two
                pieces behind the AllGather launches so h_tall[q] is ready."""
                _, qa, qb = qruns[q]
                qw = (qb - qa) * 128
                qn = qb - qa
                for s in range(s0, s1):
                    xs = slab.tile([D + 1, QMAX * 128], fp16, tag="slab2")
                    nc.sync.dma_start(xs[0:D, 0:qw],
                                      h_tall_q[q].ap()[s * D:(s + 1) * D, :])
                    nc.sync.dma_start(xs[D:D + 1, 0:qw],
                                      ones_d.ap()[:, 0:qw])
                    tb = dpool.tile([128, (NCHUNK + 2) // 3, TCOL], fp16,
                                    tag="tabs")
                    for j in range(qn):
                        ps = psum_p.tile([128, TCOL], fp32, tag="dps")
                        nc.tensor.matmul(ps[:], xs[:, j * 128:(j + 1) * 128],
                                         Wb2_tab_sb[:], start=True, stop=True)
                        if j % 2 == 0:
                            nc.scalar.activation(tb[:, j, :], ps[:], AF.Copy)
                        else:
                            nc.vector.tensor_copy(tb[:, j, :], ps[:])
                    r0 = (s * NCHUNK + qa) * 128
                    nc.sync.dma_start(
                        T_d[1].ap()[r0:r0 + qn * 128, 0:TCOL]
                        .rearrange("(b l) c -> l b c", l=128),
                        tb[:, 0:qn, :])

            import functools
            table_pass(0, x_slab)
            own_pass(0)
            load_ae(0)
            load_ae(1)
            prep_aeadst(0)
            done = stage == "dense1"
            if not done:
                # Layer-2 prep is interleaved into edge phase 1 as post-group
                # tasks: epilogue (h + own2/prep2) at each piece's end group,
                # collective launch one group later, table-2 builds two pieces
                # behind (split into slab halves across adjacent groups).
                tasks0 = {}
                post0 = []

                def _at(gi, fn):
                    if gi < NGRP:
                        tasks0.setdefault(gi, []).append(fn)
                    else:
                        post0.append(fn)

                P = functools.partial
                if stage != "edge1":
                    for q, (gi, ca, cb) in enumerate(qruns):
                        _at(gi, P(quarter_epilogue, q))
                        _at(gi + 1, P(launch_collective, q))
                    # All table-2 builds go after edge phase 1 (its DMA is
                    # saturated; the builds fill the collective-wait trough).
                    # Pieces 0..NQ-2 landed during edge 1, so these stream
                    # without stalls; build NQ-1 waits only the tiny last
                    # piece, which runs concurrently with the earlier builds.
                    for q in range(NQ - 1):
                        post0.append(P(table2_build, q, 0, NCORE))
                edge_phase(0, tasks0)
                for fn in post0:
                    fn()
                done = stage == "edge1"
            if done:
                finish_early()
            else:
                # ================= layer 2 =================
                table2_build(NQ - 1, 0, NCORE)
                if stage == "dense2":
                    finish_early()
                else:
                    tasks1 = {}
                    post1 = []
                    for q, (gi, ca, cb) in enumerate(qruns):
                        if gi + 1 < NGRP:
                            tasks1.setdefault(gi + 1, []).append(
                                functools.partial(y_quarter, q))
                        else:
                            post1.append(functools.partial(y_quarter, q))
                    edge_phase(1, tasks1)
                    for fn in post1:
                        fn()

    nc.compile()
    return nc


def _get_nc(cfg):
    import os
    stage = os.environ.get("KERNEL_STAGE", "full")
    key = (tuple(sorted(cfg.items())), stage)
    if key not in _BUILD_CACHE:
        _BUILD_CACHE[key] = _build(key[0], stage)
    return _BUILD_CACHE[key]


# ----------------------------------------------------------------------------
# Entry point
# ----------------------------------------------------------------------------

def kernel(**inputs):
    import sys
    if "/opt/trn_rl_repo" not in sys.path:
        sys.path.insert(0, "/opt/trn_rl_repo")
    from concourse.bass_utils import run_bass_kernel_spmd

    cfg, in_maps, perm = _prepare(**inputs)
    nc = _get_nc(cfg)
    res = run_bass_kernel_spmd(nc, in_maps, core_ids=list(range(NCORE)))
    kernel.last_results = res

    N, D = cfg["N"], cfg["D"]
    y = np.empty((N, D), dtype=np.float32)
    for c in range(NCORE):
        n = perm[c]
        valid = n >= 0
        y[n[valid]] = res.results[c]["y"][:valid.sum()]
    return y



# revision 23
# speedup vs baseline: 1.0074x; 1.0074x over previous
"""Trainium2 Bass kernel for a 2-layer edge-conditioned GAT (PyG GATConv style).

Strategy (8 NeuronCores, SPMD, node-parallel):
  - Nodes dealt to cores round-robin in degree order; each core owns softmax +
    aggregation for its nodes.  Per core, nodes are bucketed into 128-lane
    chunks; incoming edges form a padded [lane, slot] grid so per-edge ops are
    dense tile ops.
  - The per-layer gather table ([xl | 1 | a_src] per node, 66 fp16 cols in
    256B-strided rows) is computed REPLICATED on every core by a cheap dense
    matmul pass (layer 1 from the replicated x; layer 2 from an fp16
    AllGather of h^T).  No table AllGather.
  - Edge gathers use SWDGE dma_gather with int16 indices.  The >32768-row
    range is covered by NW=5 overlapping 32768-row windows; edges in window
    overlaps are assigned to windows so as to minimize the padded per-chunk
    grid widths (min-cost interval assignment, SPMD-uniform across cores).
  - Scores: e = lrelu(a_src[src] + a_dst[dst] + c*ea); the max-shift of the
    reference softmax is replaced by a constant shift (exact: softmax is
    shift invariant; scores are bounded).  The appended all-ones table column
    makes the softmax denominator fall out of the same fused multiply+reduce
    that aggregates features.
"""

import math

import numpy as np

NCORE = 8
ROW = 128          # fp16 elements per gather-table row (= 256B, SWDGE minimum)
TCOL = 66          # used table columns: [xl(0:64) | one | a_src]
B_MAX = 150        # max gather blocks (of 128 edges) per group
SUB_BLK = 8        # blocks per dma_gather call (1024 idxs = Q7 ucode scratch cap)
SCRATCH = 32768    # SWDGE ring carveout (2048 descriptors = 2 calls in flight)
                   # = 2 gather calls in flight: prep overlaps transfer)
LIM = 32768        # int16 gather window (rows per window)
NW = 5             # gather windows
EXP_SHIFT = -8.0   # constant softmax shift
PAD_AE = -60000.0  # score for padded slots -> exp == 0

_BUILD_CACHE = {}


# ----------------------------------------------------------------------------
# Host-side preprocessing
# ----------------------------------------------------------------------------

def _pack(src_p, dst_core, dst_chunk, dst_lane, NCHUNK, S):
    """4-window grid packing.  Returns per-chunk per-window widths W [NCHUNK,NW],
    and per-edge (window, depth) assignments.  Widths are shared across cores
    (SPMD-uniform program)."""
    EE = len(src_p)
    # window-coverage interval [lo, hi] per edge (coverage is contiguous)
    lo = np.zeros(EE, np.int8)
    hi = np.zeros(EE, np.int8)
    cov = np.zeros((NW, EE), bool)
    for w in range(NW):
        cov[w] = (src_p >= S[w]) & (src_p < S[w] + LIM)
    lo = np.argmax(cov, axis=0).astype(np.int8)
    hi = (NW - 1 - np.argmax(cov[::-1], axis=0)).astype(np.int8)
    assert (cov[lo, np.arange(EE)] & cov[hi, np.arange(EE)]).all()

    gkey = (dst_core.astype(np.int64) * NCHUNK + dst_chunk) * 128 + dst_lane
    NKEY = NCORE * NCHUNK * 128

    # per-key demand per class (class = (lo, hi) pair); classes are few
    classes = sorted({(int(a), int(b)) for a, b in zip(lo, hi)})
    cidx = {c: i for i, c in enumerate(classes)}
    ecls = np.array([cidx[(int(a), int(b))] for a, b in zip(lo, hi)],
                    dtype=np.int8)
    NCLS = len(classes)
    dem_cls = np.zeros((NKEY, NCLS), np.int32)
    np.add.at(dem_cls, (gkey, ecls), 1)

    key_chunk = (np.arange(NKEY) // 128) % NCHUNK

    # interval-constraint DP for per-chunk widths (joint across cores):
    # c[k+1] = max_i (c[i] + dem[i,k]) where dem[i,k] = max over keys of
    # edges whose interval is within [i, k]
    W = np.zeros((NCHUNK, NW), np.int64)
    for j in range(NCHUNK):
        sel = key_chunk == j
        dj = dem_cls[sel]
        dem = {}
        for i in range(NW):
            for k in range(i, NW):
                csel = [cidx[c] for c in classes
                        if c[0] >= i and c[1] <= k]
                dem[(i, k)] = int(dj[:, csel].sum(axis=1).max()) if csel else 0
        c = [0] * (NW + 1)
        for k in range(NW):
            c[k + 1] = max([c[i] + dem[(i, k)] for i in range(k + 1)] + [c[k]])
        c[1] = max(c[1], 1)
        for k in range(1, NW):
            c[k + 1] = max(c[k + 1], c[k])
        W[j] = np.diff(np.array(c))
    assert (W[:, 0] >= 1).all()

    # per-key greedy assignment: for w in 0..NW-1 take classes by ascending hi
    remaining = dem_cls.astype(np.int64).copy()
    take = np.zeros((NKEY, NCLS, NW), np.int32)   # edges of class -> window
    order = sorted(range(NCLS), key=lambda ci: (classes[ci][1], classes[ci][0]))
    for w in range(NW):
        cap = W[key_chunk, w].copy()
        for ci in order:
            clo, chi = classes[ci]
            if not (clo <= w <= chi):
                continue
            if chi == w:
                t = remaining[:, ci].copy()   # must take all
            else:
                t = np.minimum(remaining[:, ci], cap)
            take[:, ci, w] = t
            cap -= t
            remaining[:, ci] -= t
        assert (cap >= 0).all(), f"window {w} overflow"
    assert (remaining == 0).all(), "assignment infeasible"

    # per-edge window: position within (key, class) decides the window
    eorder = np.lexsort((ecls, gkey))
    kc_sorted = gkey[eorder] * np.int64(NCLS) + ecls[eorder]
    starts = np.r_[0, np.flatnonzero(np.diff(kc_sorted)) + 1]
    counts = np.diff(np.r_[starts, EE])
    posin = np.arange(EE) - np.repeat(starts, counts)
    cum = np.cumsum(take, axis=2)               # [NKEY, NCLS, NW]
    pos_e = np.empty(EE, np.int64)
    pos_e[eorder] = posin
    cum_e = cum[gkey, ecls]                     # [EE, NW]
    win_e = (pos_e[:, None] >= cum_e).sum(axis=1).astype(np.int8)
    assert (win_e < NW).all()
    assert (cov[win_e, np.arange(EE)]).all()

    # depth within (key, window)
    kw = gkey * np.int64(NW) + win_e
    o2 = np.argsort(kw, kind="stable")
    kw_s = kw[o2]
    st2 = np.r_[0, np.flatnonzero(np.diff(kw_s)) + 1]
    cn2 = np.diff(np.r_[st2, EE])
    k_s = np.arange(EE) - np.repeat(st2, cn2)
    depth_e = np.empty(EE, np.int64)
    depth_e[o2] = k_s
    assert (depth_e < W[dst_chunk, win_e]).all()
    return W, win_e, depth_e


def _prepare(x, edge_index, edge_attr,
             W_res, b_res, alpha_mix,
             W1, att_src1, att_dst1, We1, att_e1, b1,
             W2, att_src2, att_dst2, We2, att_e2, b2):
    N, D = x.shape
    E = edge_index.shape[1]
    f32 = np.float32

    src = np.concatenate([edge_index[0], np.arange(N, dtype=np.int64)]).astype(np.int64)
    dst = np.concatenate([edge_index[1], np.arange(N, dtype=np.int64)]).astype(np.int64)
    ea = np.concatenate([edge_attr[:, 0].astype(f32),
                         np.full(N, edge_attr.astype(f32).mean(), dtype=f32)])
    EE = E + N

    deg = np.bincount(dst, minlength=N)

    PCORE = int(math.ceil(N / NCORE / 128) * 128)
    NCHUNK = PCORE // 128
    TROWS = NCORE * PCORE
    B0 = TROWS - LIM
    S = tuple(int(round(i * B0 / (NW - 1))) for i in range(NW))

    # Deal nodes to cores round-robin in degree order (balances edge counts);
    # within a core sort by degree so chunk members have similar degree.
    order = np.argsort(deg, kind="stable")
    rank = np.empty(N, dtype=np.int64)
    rank[order] = np.arange(N)
    core_of = (rank % NCORE).astype(np.int32)
    pos_in_core = np.full(N, -1, dtype=np.int64)
    perm = np.full((NCORE, PCORE), -1, dtype=np.int64)
    for c in range(NCORE):
        nodes = np.where(core_of == c)[0]
        nodes = nodes[np.argsort(deg[nodes], kind="stable")]
        pos_in_core[nodes] = np.arange(len(nodes))
        perm[c, :len(nodes)] = nodes
    p_id = core_of.astype(np.int64) * PCORE + pos_in_core

    src_p = p_id[src]
    dst_core = core_of[dst]
    dst_chunk = pos_in_core[dst] // 128
    dst_lane = pos_in_core[dst] % 128

    W, win_e, depth_e = _pack(src_p, dst_core, dst_chunk, dst_lane, NCHUNK, S)

    # Greedy group packing: chunks -> groups with <= B_MAX blocks each.
    # Group packing with a ramp: small first groups so the vector engine
    # starts while later gathers stream, and a small final group so the
    # exposed tail compute is short.
    wsum = W.sum(axis=1)
    groups = []
    gs, acc = 0, 0
    for j in range(NCHUNK):
        b = int(wsum[j])
        cap = (B_MAX // 3 if len(groups) == 0 else
               2 * B_MAX // 3 if len(groups) == 1 else B_MAX)
        if acc + b > cap and j > gs:
            groups.append((gs, j))
            gs, acc = j, 0
        acc += b
    groups.append((gs, NCHUNK))
    # split an oversized tail group in half
    if len(groups) >= 2:
        a, b = groups[-1]
        if b - a >= 4 and wsum[a:b].sum() > B_MAX // 2:
            mid = a + (b - a) // 2
            groups[-1:] = [(a, mid), (mid, b)]

    # Block layout per group: [win0 blocks of its chunks | win1 | win2 | win3]
    base = np.zeros((NCHUNK, NW), np.int64)
    group_info = []   # (blk0, (nblk per window), (ca, cb))
    bpos = 0
    for (a, b) in groups:
        blk0 = bpos
        nblk = []
        for w in range(NW):
            n0 = bpos
            for j in range(a, b):
                base[j, w] = bpos
                bpos += int(W[j, w])
            nblk.append(bpos - n0)
        group_info.append((blk0, tuple(nblk), (a, b)))
    B_TOT = bpos
    NSLOT = B_TOT * 128

    blk_e = base[dst_chunk, win_e] + depth_e
    slot_e = blk_e * 128 + dst_lane
    idx_val = (src_p - np.asarray(S, dtype=np.int64)[win_e])
    assert (idx_val >= 0).all() and (idx_val < LIM).all()
    idx_val = idx_val.astype(np.int16)

    c1 = float(np.dot(We1[0].astype(f32), att_e1.astype(f32)))
    c2 = float(np.dot(We2[0].astype(f32), att_e2.astype(f32)))

    idx_imgs, ae1_imgs, ae2_imgs = [], [], []
    for c in range(NCORE):
        m = dst_core == c
        sl = slot_e[m]
        grid_idx = np.zeros(NSLOT, dtype=np.int16)
        grid_idx[sl] = idx_val[m]
        g1 = np.full(NSLOT, PAD_AE, dtype=f32)
        g1[sl] = c1 * ea[m]
        g2 = np.full(NSLOT, PAD_AE, dtype=f32)
        g2[sl] = c2 * ea[m]
        img16 = grid_idx.reshape(-1, 16).T
        # dma_gather on queue 0 reads 32 idx channels; 2x-replicated 16-wrap
        idx_imgs.append(np.tile(img16, (2, 1)).copy())
        ae1_imgs.append(np.ascontiguousarray(g1.reshape(B_TOT, 128).T.astype(np.float16)))
        ae2_imgs.append(np.ascontiguousarray(g2.reshape(B_TOT, 128).T.astype(np.float16)))

    # Give fully-padded lanes (node-count padding) one live slot so s > 0.
    lane_has = np.zeros((NCORE, PCORE), dtype=bool)
    lane_has[dst_core, pos_in_core[dst]] = True
    for c in range(NCORE):
        for j in range(NCHUNK):
            dead = np.where(~lane_has[c, j * 128:(j + 1) * 128])[0]
            if len(dead):
                ae1_imgs[c][dead, base[j, 0]] = 0.0
                ae2_imgs[c][dead, base[j, 0]] = 0.0

    # Weights.  Residual Linear folded into layer 1; biases folded via the
    # ones row of the lhsT (layer 1) / zero-bias (layer 2).
    W_res = W_res.astype(f32)
    b_res = b_res.astype(f32)
    W1 = W1.astype(f32)
    W2 = W2.astype(f32)
    alpha = float(alpha_mix)
    W1e = W_res @ W1
    b1e = b_res @ W1

    # Table pass rhs, layer 1: cols [xl(0:64) | one | a_src], lhsT rows [x; 1]
    Wb1_tab = np.zeros((D + 1, TCOL), dtype=f32)
    Wb1_tab[:D, 0:D] = W1e
    Wb1_tab[D, 0:D] = b1e
    Wb1_tab[D, D] = 1.0
    Wb1_tab[:D, D + 1] = W1e @ att_src1.astype(f32)
    Wb1_tab[D, D + 1] = float(b1e @ att_src1.astype(f32))
    # Own pass rhs, layer 1: cols [a_dst | alpha*xres(0:64)]
    Wb1_own = np.zeros((D + 1, 1 + D), dtype=f32)
    Wb1_own[:D, 0] = W1e @ att_dst1.astype(f32)
    Wb1_own[D, 0] = float(b1e @ att_dst1.astype(f32))
    Wb1_own[:D, 1:] = W_res * alpha
    Wb1_own[D, 1:] = b_res * alpha
    # Table pass rhs, layer 2 (lhsT rows [h; 1])
    Wb2_tab = np.zeros((D + 1, TCOL), dtype=f32)
    Wb2_tab[:D, 0:D] = W2
    Wb2_tab[D, D] = 1.0
    Wb2_tab[:D, D + 1] = W2 @ att_src2.astype(f32)
    # Own pass rhs, layer 2: col [a_dst2]
    Wb2_own = np.zeros((D + 1, 1), dtype=f32)
    Wb2_own[:D, 0] = W2 @ att_dst2.astype(f32)

    # Dense inputs: x^T in p_id order with ones row, fp16.
    xT_full = np.zeros((D + 1, TROWS), dtype=np.float16)
    valid_all = perm.reshape(-1) >= 0
    cols = np.arange(TROWS)[valid_all]
    xT_full[:D, cols] = x[perm.reshape(-1)[valid_all]].astype(np.float16).T
    xT_full[D, :] = 1.0

    WMAXW = int(W.max())
    WMAXC = int(W.sum(axis=1).max())

    cfg = dict(
        N=N, D=D, PCORE=PCORE, NCHUNK=NCHUNK, TROWS=TROWS,
        B_TOT=B_TOT, NSLOT=NSLOT, WMAXW=WMAXW, WMAXC=WMAXC, S=S,
        W=tuple(tuple(int(v) for v in row) for row in W),
        base=tuple(tuple(int(v) for v in row) for row in base),
        groups=tuple((int(b0), tuple(int(n) for n in nblk), (int(a), int(b)))
                     for (b0, nblk, (a, b)) in group_info),
    )

    in_maps = []
    for c in range(NCORE):
        in_maps.append(dict(
            xT_full=xT_full,
            xT_own=np.ascontiguousarray(xT_full[:, c * PCORE:(c + 1) * PCORE]),
            idx_img=idx_imgs[c],
            ae1=ae1_imgs[c],
            ae2=ae2_imgs[c],
            Wb1_tab=Wb1_tab.astype(np.float16),
            Wb1_own=Wb1_own.astype(np.float16),
            Wb2_tab=Wb2_tab.astype(np.float16),
            Wb2_own=Wb2_own.astype(np.float16),
            b1row=np.tile(b1.astype(f32).reshape(1, D), (128, 1)),
            b2row=np.tile(b2.astype(f32).reshape(1, D), (128, 1)),
            ones_row=np.ones((1, PCORE), dtype=np.float16),
            ident=np.eye(128, dtype=np.float16),
        ))
    return cfg, in_maps, perm


# ----------------------------------------------------------------------------
# Device program
# ----------------------------------------------------------------------------

def _build(cfg_key, stage='full'):
    import contextlib

    import concourse.bass as bass
    import concourse.tile as tile
    import concourse.mybir as mybir
    from concourse import bacc
    from concourse.library_config import mlp

    cfg = dict(cfg_key)
    D = cfg["D"]
    PCORE, NCHUNK, TROWS = cfg["PCORE"], cfg["NCHUNK"], cfg["TROWS"]
    B_TOT, NSLOT, WMAXW = cfg["B_TOT"], cfg["NSLOT"], cfg["WMAXW"]
    WMAXC = cfg["WMAXC"]
    S = cfg["S"]
    W = cfg["W"]
    base = cfg["base"]
    groups = cfg["groups"]

    fp16 = mybir.dt.float16
    fp32 = mybir.dt.float32
    i16 = mybir.dt.int16
    AF = mybir.ActivationFunctionType
    ALU = mybir.AluOpType

    nc = bacc.Bacc("TRN2", target_bir_lowering=False, debug=False,
                   num_devices=NCORE, dynamic_dma_scratch_size=SCRATCH)

    xT_full_d = nc.dram_tensor("xT_full", [D + 1, TROWS], fp16, kind="ExternalInput")
    xT_own_d = nc.dram_tensor("xT_own", [D + 1, PCORE], fp16, kind="ExternalInput")
    idx_img = nc.dram_tensor("idx_img", [32, NSLOT // 16], i16, kind="ExternalInput")
    ae1_d = nc.dram_tensor("ae1", [128, B_TOT], fp16, kind="ExternalInput")
    ae2_d = nc.dram_tensor("ae2", [128, B_TOT], fp16, kind="ExternalInput")
    Wb1_tab_d = nc.dram_tensor("Wb1_tab", [D + 1, TCOL], fp16, kind="ExternalInput")
    Wb1_own_d = nc.dram_tensor("Wb1_own", [D + 1, 1 + D], fp16, kind="ExternalInput")
    Wb2_tab_d = nc.dram_tensor("Wb2_tab", [D + 1, TCOL], fp16, kind="ExternalInput")
    Wb2_own_d = nc.dram_tensor("Wb2_own", [D + 1, 1], fp16, kind="ExternalInput")
    b1row_d = nc.dram_tensor("b1row", [128, D], fp32, kind="ExternalInput")
    b2row_d = nc.dram_tensor("b2row", [128, D], fp32, kind="ExternalInput")
    ones_d = nc.dram_tensor("ones_row", [1, PCORE], fp16, kind="ExternalInput")
    ident_d = nc.dram_tensor("ident", [128, 128], fp16, kind="ExternalInput")
    y_d = nc.dram_tensor("y", [PCORE, D], fp32, kind="ExternalOutput")

    T_d = [nc.dram_tensor(f"T{l}", [TROWS, ROW], fp16) for l in range(2)]

    # Piece split of the group list: the h AllGather is pipelined in NQ
    # column pieces, each issued as soon as its chunks' epilogue is done so
    # the collective overlaps the remaining edge-phase groups.
    NGRP = len(groups)
    # Skewed boundaries: the serial collective pipe fills back-to-front, so
    # early pieces are big (launched early, hidden) and the last two pieces
    # are small (their latency is what edge phase 2 waits on).
    qbound = []
    for f in (0.36, 0.64, 0.82):
        b = max(1, min(NGRP - 2, round(NGRP * f)))
        if not qbound or b > qbound[-1]:
            qbound.append(b)
    if NGRP - 1 > (qbound[-1] if qbound else 0):
        qbound.append(NGRP - 1)
    qbound.append(NGRP)
    NQ = len(qbound)
    qruns = []
    g0 = 0
    for q in range(NQ):
        g1 = qbound[q]
        ca = groups[g0][2][0]
        cb = groups[g1 - 1][2][1]
        qruns.append((g1 - 1, ca, cb))
        g0 = g1
    # h travels fp8: per-node a_dst error cancels in the segment softmax
    # (constant shift per destination), and the xl2/a_src2 error is averaged
    # over D=64 terms.  Rows [h(64) | ones] so the table-2 matmul keeps its
    # denominator column without a separate ones load.
    fp8 = mybir.dt.float8e4
    h_tin_q = [nc.dram_tensor(f"h_tin{q}", [D + 1, (cb - ca) * 128], fp8)
               for q, (_, ca, cb) in enumerate(qruns)]
    h_tall_q = [nc.dram_tensor(f"h_tall{q}", [NCORE * (D + 1), (cb - ca) * 128],
                               fp8, addr_space="Shared")
                for q, (_, ca, cb) in enumerate(qruns)]

    nc.gpsimd.load_library(mlp)
    rg = [list(range(NCORE))]

    with tile.TileContext(nc) as tc:
        with contextlib.ExitStack() as ctx:
            resident = ctx.enter_context(tc.tile_pool(name="resident", bufs=1))
            slab = ctx.enter_context(tc.tile_pool(name="slab", bufs=2))
            gpool = ctx.enter_context(tc.tile_pool(name="gather", bufs=2))
            ppool = ctx.enter_context(tc.tile_pool(name="prod", bufs=2))
            spool = ctx.enter_context(tc.tile_pool(name="small", bufs=3))
            epool = ctx.enter_context(tc.tile_pool(name="epil", bufs=2))
            dpool = ctx.enter_context(tc.tile_pool(name="dense", bufs=2))
            ipool = ctx.enter_context(tc.tile_pool(name="idx", bufs=2))
            psum_p = ctx.enter_context(tc.tile_pool(name="ps", bufs=6, space="PSUM"))
            psum_t = ctx.enter_context(tc.tile_pool(name="pst", bufs=2, space="PSUM"))

            # ---------------- resident loads ----------------
            Wb1_tab_sb = resident.tile([D + 1, TCOL], fp16)
            nc.sync.dma_start(Wb1_tab_sb[:], Wb1_tab_d.ap())
            Wb1_own_sb = resident.tile([D + 1, 1 + D], fp16)
            nc.sync.dma_start(Wb1_own_sb[:], Wb1_own_d.ap())
            Wb2_tab_sb = resident.tile([D + 1, TCOL], fp16)
            nc.sync.dma_start(Wb2_tab_sb[:], Wb2_tab_d.ap())
            Wb2_own_sb = resident.tile([D + 1, 1], fp16)
            nc.sync.dma_start(Wb2_own_sb[:], Wb2_own_d.ap())
            b1row = resident.tile([128, D], fp32)
            nc.sync.dma_start(b1row[:], b1row_d.ap())
            b2row = resident.tile([128, D], fp32)
            nc.sync.dma_start(b2row[:], b2row_d.ap())
            ident = resident.tile([128, 128], fp16)
            nc.sync.dma_start(ident[:], ident_d.ap())
            expshift = resident.tile([128, 1], fp32)
            nc.vector.memset(expshift[:], EXP_SHIFT)

            h_T = resident.tile([D + 1, PCORE], fp8)
            nc.vector.memset(h_T[D:D + 1, :], 1.0)
            Wb2_tab8 = resident.tile([D + 1, TCOL], fp8)
            nc.vector.tensor_copy(Wb2_tab8[:], Wb2_tab_sb[:])
            Wb2_own8 = resident.tile([D + 1, 1], fp8)
            nc.vector.tensor_copy(Wb2_own8[:], Wb2_own_sb[:])

            ae_sb = [resident.tile([128, B_TOT], fp16, name=f"ae_sb{l}")
                     for l in range(2)]
            aeadst = [resident.tile([128, B_TOT], fp16, name=f"aeadst{l}")
                      for l in range(2)]
            xres16 = resident.tile([128, NCHUNK * D], fp16)
            h_sb = resident.tile([128, NCHUNK * D], fp16)
            adst = [resident.tile([128, NCHUNK], fp32, name=f"adst{l}")
                    for l in range(2)]
            pre_buf = resident.tile([128, NCHUNK * D], fp32)

            def table_pass(layer, lhsT_src):
                """Write the full gather table T[layer] from dense matmuls.
                lhsT_src(s) -> loads slab s ([D+1, PCORE]) and returns tile.
                One batched DMA write per slab (per-chunk writes serialize on
                the HWDGE fixed overhead); PSUM->SBUF staging alternates
                between the Activation and Vector engines."""
                half = (NCHUNK + 2) // 3
                for s in range(NCORE):
                    xs = lhsT_src(s)
                    for j0 in range(0, NCHUNK, half):
                        j1 = min(j0 + half, NCHUNK)
                        tb = dpool.tile([128, half, TCOL], fp16, tag="tabs")
                        for j in range(j0, j1):
                            ps = psum_p.tile([128, TCOL], fp32, tag="dps")
                            nc.tensor.matmul(ps[:], xs[:, j * 128:(j + 1) * 128],
                                             (Wb1_tab_sb if layer == 0 else Wb2_tab_sb)[:],
                                             start=True, stop=True)
                            if j % 2 == 0:
                                nc.scalar.activation(tb[:, j - j0, :], ps[:], AF.Copy)
                            else:
                                nc.vector.tensor_copy(tb[:, j - j0, :], ps[:])
                        r0 = (s * NCHUNK + j0) * 128
                        nc.sync.dma_start(
                            T_d[layer].ap()[r0:r0 + (j1 - j0) * 128, 0:TCOL]
                            .rearrange("(b l) c -> l b c", l=128),
                            tb[:, 0:j1 - j0, :])

            def own_pass(layer, ja=0, jb=NCHUNK, lhsT=None):
                if layer == 0:
                    xo = slab.tile([D + 1, PCORE], fp16, tag="slab")
                    nc.sync.dma_start(xo[:], xT_own_d.ap())
                    lhsT = xo
                elif lhsT is None:
                    lhsT = h_T
                ncols = (1 + D) if layer == 0 else 1
                W_own = Wb1_own_sb if layer == 0 else Wb2_own8
                for j in range(ja, jb):
                    ps = psum_p.tile([128, TCOL], fp32, tag="dps")
                    nc.tensor.matmul(ps[:, 0:ncols], lhsT[:, j * 128:(j + 1) * 128],
                                     W_own[:], start=True, stop=True)
                    nc.vector.tensor_copy(adst[layer][:, j:j + 1], ps[:, 0:1])
                    if layer == 0:
                        nc.scalar.activation(
                            xres16[:, j * D:(j + 1) * D], ps[:, 1:1 + D], AF.Copy)

            def quarter_epilogue(q):
                """h = elu(pre + b1) for quarter q's chunks, transpose into
                h_T, write h_tin[q] and kick its AllGather.  Emitted mid
                edge-phase-1 so the collective overlaps later groups."""
                _, ca, cb = qruns[q]
                for j0 in range(ca, cb, 4):
                    j1 = min(j0 + 4, cb)
                    b0, b1_ = j0 * D, j1 * D
                    w = b1_ - b0
                    nj = j1 - j0
                    t0 = epool.tile([128, 4 * D], fp32, tag="eb0")
                    nc.vector.tensor_tensor(
                        t0[:, 0:w].rearrange("l (j c) -> l j c", c=D),
                        pre_buf[:, b0:b1_].rearrange("l (j c) -> l j c", c=D),
                        b1row[:].unsqueeze(1).broadcast_to([128, nj, D]), ALU.add)
                    mneg = epool.tile([128, 4 * D], fp32, tag="eb1")
                    nc.vector.tensor_scalar_min(mneg[:, 0:w], t0[:, 0:w], 0.0)
                    eneg = epool.tile([128, 4 * D], fp32, tag="eb2")
                    nc.scalar.activation(eneg[:, 0:w], mneg[:, 0:w], AF.Exp)
                    ppos = epool.tile([128, 4 * D], fp32, tag="eb1b")
                    nc.vector.tensor_scalar_max(ppos[:, 0:w], t0[:, 0:w], 0.0)
                    nc.vector.scalar_tensor_tensor(
                        h_sb[:, b0:b1_], eneg[:, 0:w], -1.0, ppos[:, 0:w],
                        ALU.add, ALU.add)
                for j in range(ca, cb):
                    pt = psum_t.tile([D, 128], fp16, tag="pt")
                    nc.tensor.transpose(pt[:], h_sb[:, j * D:(j + 1) * D], ident[:])
                    nc.vector.tensor_copy(h_T[0:D, j * 128:(j + 1) * 128], pt[:])

            def launch_collective(q):
                """h_tin write + AllGather for piece q.  Emitted one group
                after the epilogue compute so its sem waits don't stall the
                in-order SP/Pool queues mid-stream."""
                _, ca, cb = qruns[q]
                nc.sync.dma_start(h_tin_q[q].ap(),
                                  h_T[:, ca * 128:cb * 128])
                nc.gpsimd.collective_compute(
                    "AllGather", ALU.bypass, replica_groups=rg,
                    ins=[h_tin_q[q].ap().opt()], outs=[h_tall_q[q].ap().opt()])

            def load_ae(layer):
                nc.sync.dma_start(ae_sb[layer][:],
                                  (ae1_d if layer == 0 else ae2_d).ap())

            def prep_aeadst(layer, ja=0, jb=NCHUNK):
                """ae + a_dst per slot for chunks [ja, jb)."""
                for j in range(ja, jb):
                    for w in range(NW):
                        if W[j][w]:
                            b0 = base[j][w]
                            nc.vector.tensor_scalar_add(
                                aeadst[layer][:, b0:b0 + W[j][w]],
                                ae_sb[layer][:, b0:b0 + W[j][w]],
                                adst[layer][:, j:j + 1])

            def edge_phase(layer, tasks=None):
                table = T_d[layer]
                for gi, (blk0, nblk, (ca, cb)) in enumerate(groups):
                    bg = sum(nblk)
                    G = gpool.tile([128, B_MAX, ROW], fp16, tag="G")
                    it = ipool.tile([32, B_MAX * 8], i16, tag="it")
                    nc.sync.dma_start(it[:, 0:bg * 8],
                                      idx_img.ap()[:, blk0 * 8:(blk0 + bg) * 8])
                    off = 0
                    for w in range(NW):
                        for s0 in range(0, nblk[w], SUB_BLK):
                            nb = min(SUB_BLK, nblk[w] - s0)
                            o = off + s0
                            nc.gpsimd.dma_gather(
                                G[:, o:o + nb, :],
                                table.ap()[S[w]:S[w] + LIM, :],
                                it[:, o * 8:(o + nb) * 8], nb * 128, nb * 128, ROW)
                        off += nblk[w]
                    u = spool.tile([128, B_MAX], fp32, tag="u")
                    nc.vector.tensor_tensor(
                        u[:, 0:bg], G[:, 0:bg, D + 1:D + 2].squeeze(2),
                        aeadst[layer][:, blk0:blk0 + bg], ALU.add)
                    t = spool.tile([128, B_MAX], fp32, tag="t")
                    nc.vector.scalar_tensor_tensor(
                        t[:, 0:bg], u[:, 0:bg], 0.2, u[:, 0:bg],
                        ALU.mult, ALU.max)
                    ex = spool.tile([128, B_MAX], fp16, tag="ex")
                    nc.scalar.activation(ex[:, 0:bg], t[:, 0:bg], AF.Exp,
                                         bias=expshift[:])
                    for j in range(ca, cb):
                        # P holds the chunk's windows back to back so one
                        # reduce covers the whole neighborhood.
                        P = ppool.tile([128, WMAXC, D + 1], fp16, tag="P")
                        po = 0
                        for w in range(NW):
                            dd = W[j][w]
                            if not dd:
                                continue
                            r0 = base[j][w] - blk0
                            nc.vector.tensor_tensor(
                                P[:, po:po + dd, :], G[:, r0:r0 + dd, 0:D + 1],
                                ex[:, r0:r0 + dd].unsqueeze(2)
                                .broadcast_to([128, dd, D + 1]),
                                ALU.mult)
                            po += dd
                        acc = spool.tile([128, D + 1], fp32, tag="red")
                        nc.vector.tensor_reduce(
                            acc[:], P[:, 0:po, :].transpose([0, 2, 1]),
                            axis=mybir.AxisListType.X, op=ALU.add)
                        rs = spool.tile([128, 1], fp32, tag="rs")
                        nc.vector.reciprocal(rs[:], acc[:, D:D + 1])
                        nc.vector.tensor_scalar_mul(
                            pre_buf[:, j * D:(j + 1) * D], acc[:, 0:D], rs[:])
                    if tasks:
                        for fn in tasks.get(gi, ()):
                            fn()

            def y_quarter(q):
                """y = pre + b2 + alpha*x_res for quarter q's chunks, written
                out as soon as they are reduced (overlaps later edge-2 groups)."""
                _, ca, cb = qruns[q]
                for j0 in range(ca, cb, 4):
                    j1 = min(j0 + 4, cb)
                    b0, b1_ = j0 * D, j1 * D
                    w = b1_ - b0
                    nj = j1 - j0
                    y0 = epool.tile([128, 4 * D], fp32, tag="eb0")
                    nc.vector.tensor_tensor(
                        y0[:, 0:w].rearrange("l (j c) -> l j c", c=D),
                        pre_buf[:, b0:b1_].rearrange("l (j c) -> l j c", c=D),
                        b2row[:].unsqueeze(1).broadcast_to([128, nj, D]),
                        ALU.add)
                    y1 = epool.tile([128, 4 * D], fp32, tag="eb1")
                    nc.vector.tensor_tensor(y1[:, 0:w], y0[:, 0:w],
                                            xres16[:, b0:b1_], ALU.add)
                    nc.sync.dma_start(
                        y_d.ap().rearrange("(j l) c -> l j c", l=128)
                        [:, j0:j1, :],
                        y1[:, 0:w].rearrange("l (j c) -> l j c", c=D))

            def finish_early():
                y_stub = spool.tile([128, D], fp32, tag="ystub")
                nc.vector.memset(y_stub[:], 0.0)
                nc.sync.dma_start(y_d.ap()[0:128, :], y_stub[:])

            WFREE = NCHUNK * D
            NB = 512

            # ================= layer 1 =================
            def x_slab(s):
                xs = slab.tile([D + 1, PCORE], fp16, tag="slab")
                nc.sync.dma_start(xs[:], xT_full_d.ap()[:, s * PCORE:(s + 1) * PCORE])
                return xs

            QMAX = max(cb - ca for (_, ca, cb) in qruns)

            def table2_build(q, s0, s1):
                """Table-2 rows for piece q, source slabs [s0, s1).  Paced two
                pieces behind the AllGather launches so h_tall[q] is ready."""
                _, qa, qb = qruns[q]
                qw = (qb - qa) * 128
                qn = qb - qa
                for s in range(s0, s1):
                    xs = slab.tile([D + 1, QMAX * 128], fp8, tag="slab2")
                    nc.sync.dma_start(xs[:, 0:qw],
                                      h_tall_q[q].ap()[s * (D + 1):(s + 1) * (D + 1), :])
                    tb = dpool.tile([128, max(QMAX, (NCHUNK + 2) // 3), TCOL],
                                    fp16, tag="tabs")
                    for j in range(qn):
                        ps = psum_p.tile([128, TCOL], fp32, tag="dps")
                        nc.tensor.matmul(ps[:], xs[:, j * 128:(j + 1) * 128],
                                         Wb2_tab8[:], start=True, stop=True)
                        if j % 2 == 0:
                            nc.scalar.activation(tb[:, j, :], ps[:], AF.Copy)
                        else:
                            nc.vector.tensor_copy(tb[:, j, :], ps[:])
                    r0 = (s * NCHUNK + qa) * 128
                    nc.sync.dma_start(
                        T_d[1].ap()[r0:r0 + qn * 128, 0:TCOL]
                        .rearrange("(b l) c -> l b c", l=128),
                        tb[:, 0:qn, :])

            import functools
            table_pass(0, x_slab)
            own_pass(0)
            load_ae(0)
            load_ae(1)
            prep_aeadst(0)
            done = stage == "dense1"
            if not done:
                # Layer-2 prep is interleaved into edge phase 1 as post-group
                # tasks: epilogue (h + own2/prep2) at each piece's end group,
                # collective launch one group later, table-2 builds two pieces
                # behind (split into slab halves across adjacent groups).
                tasks0 = {}
                post0 = []

                def _at(gi, fn):
                    if gi < NGRP:
                        tasks0.setdefault(gi, []).append(fn)
                    else:
                        post0.append(fn)

                P = functools.partial
                if stage != "edge1":
                    for q, (gi, ca, cb) in enumerate(qruns):
                        _at(gi, P(quarter_epilogue, q))
                        _at(gi + 1, P(launch_collective, q))
                    # Layer-2 dense prep + all table-2 builds go after edge
                    # phase 1: its DMA is saturated, while in the collective
                    # trough DVE (own/prep) and DMA (builds) are both free.
                    # Pieces 0..NQ-2 landed during edge 1, so the builds
                    # stream without stalls; build NQ-1 waits only the tiny
                    # last piece, which overlaps the earlier builds.
                    post0.append(P(own_pass, 1))
                    post0.append(P(prep_aeadst, 1))
                    for q in range(NQ - 1):
                        post0.append(P(table2_build, q, 0, NCORE))
                edge_phase(0, tasks0)
                for fn in post0:
                    fn()
                done = stage == "edge1"
            if done:
                finish_early()
            else:
                # ================= layer 2 =================
                table2_build(NQ - 1, 0, NCORE)
                if stage == "dense2":
                    finish_early()
                else:
                    tasks1 = {}
                    post1 = []
                    for q, (gi, ca, cb) in enumerate(qruns):
                        if gi + 1 < NGRP:
                            tasks1.setdefault(gi + 1, []).append(
                                functools.partial(y_quarter, q))
                        else:
                            post1.append(functools.partial(y_quarter, q))
                    edge_phase(1, tasks1)
                    for fn in post1:
                        fn()

    nc.compile()
    return nc


def _get_nc(cfg):
    import os
    stage = os.environ.get("KERNEL_STAGE", "full")
    key = (tuple(sorted(cfg.items())), stage)
    if key not in _BUILD_CACHE:
        _BUILD_CACHE[key] = _build(key[0], stage)
    return _BUILD_CACHE[key]


# ----------------------------------------------------------------------------
# Entry point
# ----------------------------------------------------------------------------

def kernel(**inputs):
    import sys
    if "/opt/trn_rl_repo" not in sys.path:
        sys.path.insert(0, "/opt/trn_rl_repo")
    from concourse.bass_utils import run_bass_kernel_spmd

    cfg, in_maps, perm = _prepare(**inputs)
    nc = _get_nc(cfg)
    res = run_bass_kernel_spmd(nc, in_maps, core_ids=list(range(NCORE)))
    kernel.last_results = res

    N, D = cfg["N"], cfg["D"]
    y = np.empty((N, D), dtype=np.float32)
    for c in range(NCORE):
        n = perm[c]
        valid = n >= 0
        y[n[valid]] = res.results[c]["y"][:valid.sum()]
    return y



# revision 28
# speedup vs baseline: 1.0081x; 1.0006x over previous
"""Trainium2 Bass kernel for a 2-layer edge-conditioned GAT (PyG GATConv style).

Strategy (8 NeuronCores, SPMD, node-parallel):
  - Nodes dealt to cores round-robin in degree order; each core owns softmax +
    aggregation for its nodes.  Per core, nodes are bucketed into 128-lane
    chunks; incoming edges form a padded [lane, slot] grid so per-edge ops are
    dense tile ops.
  - The per-layer gather table ([xl | 1 | a_src] per node, 66 fp16 cols in
    256B-strided rows) is computed REPLICATED on every core by a cheap dense
    matmul pass (layer 1 from the replicated x; layer 2 from an fp16
    AllGather of h^T).  No table AllGather.
  - Edge gathers use SWDGE dma_gather with int16 indices.  The >32768-row
    range is covered by NW=5 overlapping 32768-row windows; edges in window
    overlaps are assigned to windows so as to minimize the padded per-chunk
    grid widths (min-cost interval assignment, SPMD-uniform across cores).
  - Scores: e = lrelu(a_src[src] + a_dst[dst] + c*ea); the max-shift of the
    reference softmax is replaced by a constant shift (exact: softmax is
    shift invariant; scores are bounded).  The appended all-ones table column
    makes the softmax denominator fall out of the same fused multiply+reduce
    that aggregates features.
"""

import math

import numpy as np

NCORE = 8
ROW = 128          # fp16 elements per gather-table row (= 256B, SWDGE minimum)
TCOL = 66          # used table columns: [xl(0:64) | one | a_src]
B_MAX = 144        # max gather blocks (of 128 edges) per group
SUB_BLK = 8        # blocks per dma_gather call (1024 idxs = Q7 ucode scratch cap)
SCRATCH = 32768    # SWDGE ring carveout (2048 descriptors = 2 calls in flight)
                   # = 2 gather calls in flight: prep overlaps transfer)
LIM = 32768        # int16 gather window (rows per window)
NW = 5             # gather windows
EXP_SHIFT = -8.0   # constant softmax shift
PAD_AE = -60000.0  # score for padded slots -> exp == 0

_BUILD_CACHE = {}


# ----------------------------------------------------------------------------
# Host-side preprocessing
# ----------------------------------------------------------------------------

def _pack(src_p, dst_core, dst_chunk, dst_lane, NCHUNK, S):
    """4-window grid packing.  Returns per-chunk per-window widths W [NCHUNK,NW],
    and per-edge (window, depth) assignments.  Widths are shared across cores
    (SPMD-uniform program)."""
    EE = len(src_p)
    # window-coverage interval [lo, hi] per edge (coverage is contiguous)
    lo = np.zeros(EE, np.int8)
    hi = np.zeros(EE, np.int8)
    cov = np.zeros((NW, EE), bool)
    for w in range(NW):
        cov[w] = (src_p >= S[w]) & (src_p < S[w] + LIM)
    lo = np.argmax(cov, axis=0).astype(np.int8)
    hi = (NW - 1 - np.argmax(cov[::-1], axis=0)).astype(np.int8)
    assert (cov[lo, np.arange(EE)] & cov[hi, np.arange(EE)]).all()

    gkey = (dst_core.astype(np.int64) * NCHUNK + dst_chunk) * 128 + dst_lane
    NKEY = NCORE * NCHUNK * 128

    # per-key demand per class (class = (lo, hi) pair); classes are few
    classes = sorted({(int(a), int(b)) for a, b in zip(lo, hi)})
    cidx = {c: i for i, c in enumerate(classes)}
    ecls = np.array([cidx[(int(a), int(b))] for a, b in zip(lo, hi)],
                    dtype=np.int8)
    NCLS = len(classes)
    dem_cls = np.zeros((NKEY, NCLS), np.int32)
    np.add.at(dem_cls, (gkey, ecls), 1)

    key_chunk = (np.arange(NKEY) // 128) % NCHUNK

    # interval-constraint DP for per-chunk widths (joint across cores):
    # c[k+1] = max_i (c[i] + dem[i,k]) where dem[i,k] = max over keys of
    # edges whose interval is within [i, k]
    W = np.zeros((NCHUNK, NW), np.int64)
    for j in range(NCHUNK):
        sel = key_chunk == j
        dj = dem_cls[sel]
        dem = {}
        for i in range(NW):
            for k in range(i, NW):
                csel = [cidx[c] for c in classes
                        if c[0] >= i and c[1] <= k]
                dem[(i, k)] = int(dj[:, csel].sum(axis=1).max()) if csel else 0
        c = [0] * (NW + 1)
        for k in range(NW):
            c[k + 1] = max([c[i] + dem[(i, k)] for i in range(k + 1)] + [c[k]])
        c[1] = max(c[1], 1)
        for k in range(1, NW):
            c[k + 1] = max(c[k + 1], c[k])
        W[j] = np.diff(np.array(c))
    assert (W[:, 0] >= 1).all()

    # per-key greedy assignment: for w in 0..NW-1 take classes by ascending hi
    remaining = dem_cls.astype(np.int64).copy()
    take = np.zeros((NKEY, NCLS, NW), np.int32)   # edges of class -> window
    order = sorted(range(NCLS), key=lambda ci: (classes[ci][1], classes[ci][0]))
    for w in range(NW):
        cap = W[key_chunk, w].copy()
        for ci in order:
            clo, chi = classes[ci]
            if not (clo <= w <= chi):
                continue
            if chi == w:
                t = remaining[:, ci].copy()   # must take all
            else:
                t = np.minimum(remaining[:, ci], cap)
            take[:, ci, w] = t
            cap -= t
            remaining[:, ci] -= t
        assert (cap >= 0).all(), f"window {w} overflow"
    assert (remaining == 0).all(), "assignment infeasible"

    # per-edge window: position within (key, class) decides the window
    eorder = np.lexsort((ecls, gkey))
    kc_sorted = gkey[eorder] * np.int64(NCLS) + ecls[eorder]
    starts = np.r_[0, np.flatnonzero(np.diff(kc_sorted)) + 1]
    counts = np.diff(np.r_[starts, EE])
    posin = np.arange(EE) - np.repeat(starts, counts)
    cum = np.cumsum(take, axis=2)               # [NKEY, NCLS, NW]
    pos_e = np.empty(EE, np.int64)
    pos_e[eorder] = posin
    cum_e = cum[gkey, ecls]                     # [EE, NW]
    win_e = (pos_e[:, None] >= cum_e).sum(axis=1).astype(np.int8)
    assert (win_e < NW).all()
    assert (cov[win_e, np.arange(EE)]).all()

    # depth within (key, window)
    kw = gkey * np.int64(NW) + win_e
    o2 = np.argsort(kw, kind="stable")
    kw_s = kw[o2]
    st2 = np.r_[0, np.flatnonzero(np.diff(kw_s)) + 1]
    cn2 = np.diff(np.r_[st2, EE])
    k_s = np.arange(EE) - np.repeat(st2, cn2)
    depth_e = np.empty(EE, np.int64)
    depth_e[o2] = k_s
    assert (depth_e < W[dst_chunk, win_e]).all()
    return W, win_e, depth_e


def _prepare(x, edge_index, edge_attr,
             W_res, b_res, alpha_mix,
             W1, att_src1, att_dst1, We1, att_e1, b1,
             W2, att_src2, att_dst2, We2, att_e2, b2):
    N, D = x.shape
    E = edge_index.shape[1]
    f32 = np.float32

    src = np.concatenate([edge_index[0], np.arange(N, dtype=np.int64)]).astype(np.int64)
    dst = np.concatenate([edge_index[1], np.arange(N, dtype=np.int64)]).astype(np.int64)
    ea = np.concatenate([edge_attr[:, 0].astype(f32),
                         np.full(N, edge_attr.astype(f32).mean(), dtype=f32)])
    EE = E + N

    deg = np.bincount(dst, minlength=N)

    PCORE = int(math.ceil(N / NCORE / 128) * 128)
    NCHUNK = PCORE // 128
    TROWS = NCORE * PCORE
    B0 = TROWS - LIM
    S = tuple(int(round(i * B0 / (NW - 1))) for i in range(NW))

    # Deal nodes to cores round-robin in degree order (balances edge counts);
    # within a core sort by degree so chunk members have similar degree.
    order = np.argsort(deg, kind="stable")
    rank = np.empty(N, dtype=np.int64)
    rank[order] = np.arange(N)
    core_of = (rank % NCORE).astype(np.int32)
    pos_in_core = np.full(N, -1, dtype=np.int64)
    perm = np.full((NCORE, PCORE), -1, dtype=np.int64)
    for c in range(NCORE):
        nodes = np.where(core_of == c)[0]
        nodes = nodes[np.argsort(deg[nodes], kind="stable")]
        pos_in_core[nodes] = np.arange(len(nodes))
        perm[c, :len(nodes)] = nodes
    p_id = core_of.astype(np.int64) * PCORE + pos_in_core

    src_p = p_id[src]
    dst_core = core_of[dst]
    dst_chunk = pos_in_core[dst] // 128
    dst_lane = pos_in_core[dst] % 128

    W, win_e, depth_e = _pack(src_p, dst_core, dst_chunk, dst_lane, NCHUNK, S)

    # Greedy group packing: chunks -> groups with <= B_MAX blocks each.
    # Group packing with a ramp: small first groups so the vector engine
    # starts while later gathers stream, and a small final group so the
    # exposed tail compute is short.
    wsum = W.sum(axis=1)
    groups = []
    gs, acc = 0, 0
    for j in range(NCHUNK):
        b = int(wsum[j])
        cap = (B_MAX // 3 if len(groups) == 0 else
               2 * B_MAX // 3 if len(groups) == 1 else B_MAX)
        if acc + b > cap and j > gs:
            groups.append((gs, j))
            gs, acc = j, 0
        acc += b
    groups.append((gs, NCHUNK))
    # Down-ramp the tail: the final group's gather + reduce + epilogue +
    # AllGather piece is the exposed critical path into edge phase 2, so
    # split trailing groups until the last one is small.
    while True:
        a, b = groups[-1]
        if b - a < 2 or wsum[a:b].sum() <= B_MAX // 3:
            break
        mid = a + (b - a) * 2 // 3
        if mid == a:
            break
        groups[-1:] = [(a, mid), (mid, b)]

    # Block layout per group: [win0 blocks of its chunks | win1 | win2 | win3]
    base = np.zeros((NCHUNK, NW), np.int64)
    group_info = []   # (blk0, (nblk per window), (ca, cb))
    bpos = 0
    for (a, b) in groups:
        blk0 = bpos
        nblk = []
        for w in range(NW):
            n0 = bpos
            for j in range(a, b):
                base[j, w] = bpos
                bpos += int(W[j, w])
            nblk.append(bpos - n0)
        group_info.append((blk0, tuple(nblk), (a, b)))
    B_TOT = bpos
    NSLOT = B_TOT * 128

    blk_e = base[dst_chunk, win_e] + depth_e
    slot_e = blk_e * 128 + dst_lane
    idx_val = (src_p - np.asarray(S, dtype=np.int64)[win_e])
    assert (idx_val >= 0).all() and (idx_val < LIM).all()
    idx_val = idx_val.astype(np.int16)

    c1 = float(np.dot(We1[0].astype(f32), att_e1.astype(f32)))
    c2 = float(np.dot(We2[0].astype(f32), att_e2.astype(f32)))

    idx_imgs, ae1_imgs, ae2_imgs = [], [], []
    for c in range(NCORE):
        m = dst_core == c
        sl = slot_e[m]
        grid_idx = np.zeros(NSLOT, dtype=np.int16)
        grid_idx[sl] = idx_val[m]
        g1 = np.full(NSLOT, PAD_AE, dtype=f32)
        g1[sl] = c1 * ea[m]
        g2 = np.full(NSLOT, PAD_AE, dtype=f32)
        g2[sl] = c2 * ea[m]
        img16 = grid_idx.reshape(-1, 16).T
        # dma_gather on queue 0 reads 32 idx channels; 2x-replicated 16-wrap
        idx_imgs.append(np.tile(img16, (2, 1)).copy())
        ae1_imgs.append(np.ascontiguousarray(g1.reshape(B_TOT, 128).T.astype(np.float16)))
        ae2_imgs.append(np.ascontiguousarray(g2.reshape(B_TOT, 128).T.astype(np.float16)))

    # Give fully-padded lanes (node-count padding) one live slot so s > 0.
    lane_has = np.zeros((NCORE, PCORE), dtype=bool)
    lane_has[dst_core, pos_in_core[dst]] = True
    for c in range(NCORE):
        for j in range(NCHUNK):
            dead = np.where(~lane_has[c, j * 128:(j + 1) * 128])[0]
            if len(dead):
                ae1_imgs[c][dead, base[j, 0]] = 0.0
                ae2_imgs[c][dead, base[j, 0]] = 0.0

    # Weights.  Residual Linear folded into layer 1; biases folded via the
    # ones row of the lhsT (layer 1) / zero-bias (layer 2).
    W_res = W_res.astype(f32)
    b_res = b_res.astype(f32)
    W1 = W1.astype(f32)
    W2 = W2.astype(f32)
    alpha = float(alpha_mix)
    W1e = W_res @ W1
    b1e = b_res @ W1

    # Table pass rhs, layer 1: cols [xl(0:64) | one | a_src], lhsT rows [x; 1]
    Wb1_tab = np.zeros((D + 1, TCOL), dtype=f32)
    Wb1_tab[:D, 0:D] = W1e
    Wb1_tab[D, 0:D] = b1e
    Wb1_tab[D, D] = 1.0
    Wb1_tab[:D, D + 1] = W1e @ att_src1.astype(f32)
    Wb1_tab[D, D + 1] = float(b1e @ att_src1.astype(f32))
    # Own pass rhs, layer 1: cols [a_dst | alpha*xres(0:64)]
    Wb1_own = np.zeros((D + 1, 1 + D), dtype=f32)
    Wb1_own[:D, 0] = W1e @ att_dst1.astype(f32)
    Wb1_own[D, 0] = float(b1e @ att_dst1.astype(f32))
    Wb1_own[:D, 1:] = W_res * alpha
    Wb1_own[D, 1:] = b_res * alpha
    # Table pass rhs, layer 2 (lhsT rows [h; 1])
    Wb2_tab = np.zeros((D + 1, TCOL), dtype=f32)
    Wb2_tab[:D, 0:D] = W2
    Wb2_tab[D, D] = 1.0
    Wb2_tab[:D, D + 1] = W2 @ att_src2.astype(f32)
    # Own pass rhs, layer 2: col [a_dst2]
    Wb2_own = np.zeros((D + 1, 1), dtype=f32)
    Wb2_own[:D, 0] = W2 @ att_dst2.astype(f32)

    # Dense inputs: x^T in p_id order with ones row, fp16.
    xT_full = np.zeros((D + 1, TROWS), dtype=np.float16)
    valid_all = perm.reshape(-1) >= 0
    cols = np.arange(TROWS)[valid_all]
    xT_full[:D, cols] = x[perm.reshape(-1)[valid_all]].astype(np.float16).T
    xT_full[D, :] = 1.0

    WMAXW = int(W.max())
    WMAXC = int(W.sum(axis=1).max())

    cfg = dict(
        N=N, D=D, PCORE=PCORE, NCHUNK=NCHUNK, TROWS=TROWS,
        B_TOT=B_TOT, NSLOT=NSLOT, WMAXW=WMAXW, WMAXC=WMAXC, S=S,
        W=tuple(tuple(int(v) for v in row) for row in W),
        base=tuple(tuple(int(v) for v in row) for row in base),
        groups=tuple((int(b0), tuple(int(n) for n in nblk), (int(a), int(b)))
                     for (b0, nblk, (a, b)) in group_info),
    )

    in_maps = []
    for c in range(NCORE):
        in_maps.append(dict(
            xT_full=xT_full,
            xT_own=np.ascontiguousarray(xT_full[:, c * PCORE:(c + 1) * PCORE]),
            idx_img=idx_imgs[c],
            ae1=ae1_imgs[c],
            ae2=ae2_imgs[c],
            Wb1_tab=Wb1_tab.astype(np.float16),
            Wb1_own=Wb1_own.astype(np.float16),
            Wb2_tab=Wb2_tab.astype(np.float16),
            Wb2_own=Wb2_own.astype(np.float16),
            b1row=np.tile(b1.astype(f32).reshape(1, D), (128, 1)),
            b2row=np.tile(b2.astype(f32).reshape(1, D), (128, 1)),
            ones_row=np.ones((1, PCORE), dtype=np.float16),
            ident=np.eye(128, dtype=np.float16),
        ))
    return cfg, in_maps, perm


# ----------------------------------------------------------------------------
# Device program
# ----------------------------------------------------------------------------

def _build(cfg_key, stage='full'):
    import contextlib

    import concourse.bass as bass
    import concourse.tile as tile
    import concourse.mybir as mybir
    from concourse import bacc
    from concourse.library_config import mlp

    cfg = dict(cfg_key)
    D = cfg["D"]
    PCORE, NCHUNK, TROWS = cfg["PCORE"], cfg["NCHUNK"], cfg["TROWS"]
    B_TOT, NSLOT, WMAXW = cfg["B_TOT"], cfg["NSLOT"], cfg["WMAXW"]
    WMAXC = cfg["WMAXC"]
    S = cfg["S"]
    W = cfg["W"]
    base = cfg["base"]
    groups = cfg["groups"]

    fp16 = mybir.dt.float16
    fp32 = mybir.dt.float32
    i16 = mybir.dt.int16
    AF = mybir.ActivationFunctionType
    ALU = mybir.AluOpType

    nc = bacc.Bacc("TRN2", target_bir_lowering=False, debug=False,
                   num_devices=NCORE, dynamic_dma_scratch_size=SCRATCH)

    xT_full_d = nc.dram_tensor("xT_full", [D + 1, TROWS], fp16, kind="ExternalInput")
    xT_own_d = nc.dram_tensor("xT_own", [D + 1, PCORE], fp16, kind="ExternalInput")
    idx_img = nc.dram_tensor("idx_img", [32, NSLOT // 16], i16, kind="ExternalInput")
    ae1_d = nc.dram_tensor("ae1", [128, B_TOT], fp16, kind="ExternalInput")
    ae2_d = nc.dram_tensor("ae2", [128, B_TOT], fp16, kind="ExternalInput")
    Wb1_tab_d = nc.dram_tensor("Wb1_tab", [D + 1, TCOL], fp16, kind="ExternalInput")
    Wb1_own_d = nc.dram_tensor("Wb1_own", [D + 1, 1 + D], fp16, kind="ExternalInput")
    Wb2_tab_d = nc.dram_tensor("Wb2_tab", [D + 1, TCOL], fp16, kind="ExternalInput")
    Wb2_own_d = nc.dram_tensor("Wb2_own", [D + 1, 1], fp16, kind="ExternalInput")
    b1row_d = nc.dram_tensor("b1row", [128, D], fp32, kind="ExternalInput")
    b2row_d = nc.dram_tensor("b2row", [128, D], fp32, kind="ExternalInput")
    ones_d = nc.dram_tensor("ones_row", [1, PCORE], fp16, kind="ExternalInput")
    ident_d = nc.dram_tensor("ident", [128, 128], fp16, kind="ExternalInput")
    y_d = nc.dram_tensor("y", [PCORE, D], fp32, kind="ExternalOutput")

    T_d = [nc.dram_tensor(f"T{l}", [TROWS, ROW], fp16) for l in range(2)]

    # Piece split of the group list: the h AllGather is pipelined in NQ
    # column pieces, each issued as soon as its chunks' epilogue is done so
    # the collective overlaps the remaining edge-phase groups.
    NGRP = len(groups)
    # ~2 groups per piece so the serial collective pipe starts early and
    # never stacks; the last piece is the (small) final group alone, since
    # its latency is what edge phase 2 waits on.
    qbound = list(range(2, NGRP - 1, 2))
    if NGRP - 1 not in qbound:
        qbound.append(NGRP - 1)
    qbound.append(NGRP)
    NQ = len(qbound)
    qruns = []
    g0 = 0
    for q in range(NQ):
        g1 = qbound[q]
        ca = groups[g0][2][0]
        cb = groups[g1 - 1][2][1]
        qruns.append((g1 - 1, ca, cb))
        g0 = g1
    # h travels fp8: per-node a_dst error cancels in the segment softmax
    # (constant shift per destination), and the xl2/a_src2 error is averaged
    # over D=64 terms.  Rows [h(64) | ones] so the table-2 matmul keeps its
    # denominator column without a separate ones load.
    fp8 = mybir.dt.float8e4
    h_tin_q = [nc.dram_tensor(f"h_tin{q}", [D + 1, (cb - ca) * 128], fp8)
               for q, (_, ca, cb) in enumerate(qruns)]
    h_tall_q = [nc.dram_tensor(f"h_tall{q}", [NCORE * (D + 1), (cb - ca) * 128],
                               fp8, addr_space="Shared")
                for q, (_, ca, cb) in enumerate(qruns)]

    nc.gpsimd.load_library(mlp)
    rg = [list(range(NCORE))]

    with tile.TileContext(nc) as tc:
        with contextlib.ExitStack() as ctx:
            resident = ctx.enter_context(tc.tile_pool(name="resident", bufs=1))
            slab = ctx.enter_context(tc.tile_pool(name="slab", bufs=2))
            gpool = ctx.enter_context(tc.tile_pool(name="gather", bufs=2))
            ppool = ctx.enter_context(tc.tile_pool(name="prod", bufs=2))
            spool = ctx.enter_context(tc.tile_pool(name="small", bufs=3))
            epool = ctx.enter_context(tc.tile_pool(name="epil", bufs=2))
            dpool = ctx.enter_context(tc.tile_pool(name="dense", bufs=2))
            ipool = ctx.enter_context(tc.tile_pool(name="idx", bufs=3))
            psum_p = ctx.enter_context(tc.tile_pool(name="ps", bufs=6, space="PSUM"))
            psum_t = ctx.enter_context(tc.tile_pool(name="pst", bufs=2, space="PSUM"))

            # ---------------- resident loads ----------------
            Wb1_tab_sb = resident.tile([D + 1, TCOL], fp16)
            nc.sync.dma_start(Wb1_tab_sb[:], Wb1_tab_d.ap())
            Wb1_own_sb = resident.tile([D + 1, 1 + D], fp16)
            nc.sync.dma_start(Wb1_own_sb[:], Wb1_own_d.ap())
            Wb2_tab_sb = resident.tile([D + 1, TCOL], fp16)
            nc.sync.dma_start(Wb2_tab_sb[:], Wb2_tab_d.ap())
            Wb2_own_sb = resident.tile([D + 1, 1], fp16)
            nc.sync.dma_start(Wb2_own_sb[:], Wb2_own_d.ap())
            b1row = resident.tile([128, D], fp32)
            nc.sync.dma_start(b1row[:], b1row_d.ap())
            b2row = resident.tile([128, D], fp32)
            nc.sync.dma_start(b2row[:], b2row_d.ap())
            ident = resident.tile([128, 128], fp16)
            nc.sync.dma_start(ident[:], ident_d.ap())
            expshift = resident.tile([128, 1], fp32)
            nc.vector.memset(expshift[:], EXP_SHIFT)

            h_T = resident.tile([D + 1, PCORE], fp8)
            nc.vector.memset(h_T[D:D + 1, :], 1.0)
            Wb2_tab8 = resident.tile([D + 1, TCOL], fp8)
            nc.vector.tensor_copy(Wb2_tab8[:], Wb2_tab_sb[:])
            Wb2_own8 = resident.tile([D + 1, 1], fp8)
            nc.vector.tensor_copy(Wb2_own8[:], Wb2_own_sb[:])

            ae_sb = [resident.tile([128, B_TOT], fp16, name=f"ae_sb{l}")
                     for l in range(2)]
            aeadst = [resident.tile([128, B_TOT], fp16, name=f"aeadst{l}")
                      for l in range(2)]
            xres16 = resident.tile([128, NCHUNK * D], fp16)
            h_sb = resident.tile([128, NCHUNK * D], fp16)
            adst = [resident.tile([128, NCHUNK], fp32, name=f"adst{l}")
                    for l in range(2)]
            pre_buf = resident.tile([128, NCHUNK * D], fp32)

            def table_pass(layer, lhsT_src):
                """Write the full gather table T[layer] from dense matmuls.
                lhsT_src(s) -> loads slab s ([D+1, PCORE]) and returns tile.
                One batched DMA write per slab (per-chunk writes serialize on
                the HWDGE fixed overhead); PSUM->SBUF staging alternates
                between the Activation and Vector engines."""
                half = (NCHUNK + 2) // 3
                for s in range(NCORE):
                    xs = lhsT_src(s)
                    for j0 in range(0, NCHUNK, half):
                        j1 = min(j0 + half, NCHUNK)
                        tb = dpool.tile([128, half, TCOL], fp16, tag="tabs")
                        for j in range(j0, j1):
                            ps = psum_p.tile([128, TCOL], fp32, tag="dps")
                            nc.tensor.matmul(ps[:], xs[:, j * 128:(j + 1) * 128],
                                             (Wb1_tab_sb if layer == 0 else Wb2_tab_sb)[:],
                                             start=True, stop=True)
                            if j % 2 == 0:
                                nc.scalar.activation(tb[:, j - j0, :], ps[:], AF.Copy)
                            else:
                                nc.vector.tensor_copy(tb[:, j - j0, :], ps[:])
                        r0 = (s * NCHUNK + j0) * 128
                        nc.sync.dma_start(
                            T_d[layer].ap()[r0:r0 + (j1 - j0) * 128, 0:TCOL]
                            .rearrange("(b l) c -> l b c", l=128),
                            tb[:, 0:j1 - j0, :])

            def own_pass(layer, ja=0, jb=NCHUNK, lhsT=None):
                if layer == 0:
                    xo = slab.tile([D + 1, PCORE], fp16, tag="slab")
                    nc.sync.dma_start(xo[:], xT_own_d.ap())
                    lhsT = xo
                elif lhsT is None:
                    lhsT = h_T
                ncols = (1 + D) if layer == 0 else 1
                W_own = Wb1_own_sb if layer == 0 else Wb2_own8
                for j in range(ja, jb):
                    ps = psum_p.tile([128, TCOL], fp32, tag="dps")
                    nc.tensor.matmul(ps[:, 0:ncols], lhsT[:, j * 128:(j + 1) * 128],
                                     W_own[:], start=True, stop=True)
                    nc.vector.tensor_copy(adst[layer][:, j:j + 1], ps[:, 0:1])
                    if layer == 0:
                        nc.scalar.activation(
                            xres16[:, j * D:(j + 1) * D], ps[:, 1:1 + D], AF.Copy)

            def quarter_epilogue(q):
                """h = elu(pre + b1) for quarter q's chunks, transpose into
                h_T, write h_tin[q] and kick its AllGather.  Emitted mid
                edge-phase-1 so the collective overlaps later groups."""
                _, ca, cb = qruns[q]
                for j0 in range(ca, cb, 4):
                    j1 = min(j0 + 4, cb)
                    b0, b1_ = j0 * D, j1 * D
                    w = b1_ - b0
                    nj = j1 - j0
                    t0 = epool.tile([128, 4 * D], fp32, tag="eb0")
                    nc.vector.tensor_tensor(
                        t0[:, 0:w].rearrange("l (j c) -> l j c", c=D),
                        pre_buf[:, b0:b1_].rearrange("l (j c) -> l j c", c=D),
                        b1row[:].unsqueeze(1).broadcast_to([128, nj, D]), ALU.add)
                    mneg = epool.tile([128, 4 * D], fp32, tag="eb1")
                    nc.vector.tensor_scalar_min(mneg[:, 0:w], t0[:, 0:w], 0.0)
                    eneg = epool.tile([128, 4 * D], fp32, tag="eb2")
                    nc.scalar.activation(eneg[:, 0:w], mneg[:, 0:w], AF.Exp)
                    ppos = epool.tile([128, 4 * D], fp32, tag="eb1b")
                    nc.vector.tensor_scalar_max(ppos[:, 0:w], t0[:, 0:w], 0.0)
                    nc.vector.scalar_tensor_tensor(
                        h_sb[:, b0:b1_], eneg[:, 0:w], -1.0, ppos[:, 0:w],
                        ALU.add, ALU.add)
                for j in range(ca, cb):
                    pt = psum_t.tile([D, 128], fp16, tag="pt")
                    nc.tensor.transpose(pt[:], h_sb[:, j * D:(j + 1) * D], ident[:])
                    nc.vector.tensor_copy(h_T[0:D, j * 128:(j + 1) * 128], pt[:])

            def launch_collective(q):
                """h_tin write + AllGather for piece q.  Emitted one group
                after the epilogue compute so its sem waits don't stall the
                in-order SP/Pool queues mid-stream."""
                _, ca, cb = qruns[q]
                nc.sync.dma_start(h_tin_q[q].ap(),
                                  h_T[:, ca * 128:cb * 128])
                nc.gpsimd.collective_compute(
                    "AllGather", ALU.bypass, replica_groups=rg,
                    ins=[h_tin_q[q].ap().opt()], outs=[h_tall_q[q].ap().opt()])

            def load_ae(layer):
                nc.sync.dma_start(ae_sb[layer][:],
                                  (ae1_d if layer == 0 else ae2_d).ap())

            def prep_aeadst(layer, ja=0, jb=NCHUNK):
                """ae + a_dst per slot for chunks [ja, jb)."""
                for j in range(ja, jb):
                    for w in range(NW):
                        if W[j][w]:
                            b0 = base[j][w]
                            nc.vector.tensor_scalar_add(
                                aeadst[layer][:, b0:b0 + W[j][w]],
                                ae_sb[layer][:, b0:b0 + W[j][w]],
                                adst[layer][:, j:j + 1])

            def edge_phase(layer, tasks=None):
                table = T_d[layer]
                for gi, (blk0, nblk, (ca, cb)) in enumerate(groups):
                    bg = sum(nblk)
                    G = gpool.tile([128, B_MAX, ROW], fp16, tag="G")
                    it = ipool.tile([32, B_MAX * 8], i16, tag="it")
                    nc.sync.dma_start(it[:, 0:bg * 8],
                                      idx_img.ap()[:, blk0 * 8:(blk0 + bg) * 8])
                    off = 0
                    for w in range(NW):
                        for s0 in range(0, nblk[w], SUB_BLK):
                            nb = min(SUB_BLK, nblk[w] - s0)
                            o = off + s0
                            nc.gpsimd.dma_gather(
                                G[:, o:o + nb, :],
                                table.ap()[S[w]:S[w] + LIM, :],
                                it[:, o * 8:(o + nb) * 8], nb * 128, nb * 128, ROW)
                        off += nblk[w]
                    u = spool.tile([128, B_MAX], fp32, tag="u")
                    nc.vector.tensor_tensor(
                        u[:, 0:bg], G[:, 0:bg, D + 1:D + 2].squeeze(2),
                        aeadst[layer][:, blk0:blk0 + bg], ALU.add)
                    t = spool.tile([128, B_MAX], fp32, tag="t")
                    nc.vector.scalar_tensor_tensor(
                        t[:, 0:bg], u[:, 0:bg], 0.2, u[:, 0:bg],
                        ALU.mult, ALU.max)
                    ex = spool.tile([128, B_MAX], fp16, tag="ex")
                    nc.scalar.activation(ex[:, 0:bg], t[:, 0:bg], AF.Exp,
                                         bias=expshift[:])
                    for j in range(ca, cb):
                        # P holds the chunk's windows back to back so one
                        # reduce covers the whole neighborhood.
                        P = ppool.tile([128, WMAXC, D + 1], fp16, tag="P")
                        po = 0
                        for w in range(NW):
                            dd = W[j][w]
                            if not dd:
                                continue
                            r0 = base[j][w] - blk0
                            nc.vector.tensor_tensor(
                                P[:, po:po + dd, :], G[:, r0:r0 + dd, 0:D + 1],
                                ex[:, r0:r0 + dd].unsqueeze(2)
                                .broadcast_to([128, dd, D + 1]),
                                ALU.mult)
                            po += dd
                        acc = spool.tile([128, D + 1], fp32, tag="red")
                        nc.vector.tensor_reduce(
                            acc[:], P[:, 0:po, :].transpose([0, 2, 1]),
                            axis=mybir.AxisListType.X, op=ALU.add)
                        rs = spool.tile([128, 1], fp32, tag="rs")
                        nc.vector.reciprocal(rs[:], acc[:, D:D + 1])
                        nc.vector.tensor_scalar_mul(
                            pre_buf[:, j * D:(j + 1) * D], acc[:, 0:D], rs[:])
                    if tasks:
                        for fn in tasks.get(gi, ()):
                            fn()

            def y_quarter(q):
                """y = pre + b2 + alpha*x_res for quarter q's chunks, written
                out as soon as they are reduced (overlaps later edge-2 groups)."""
                _, ca, cb = qruns[q]
                for j0 in range(ca, cb, 4):
                    j1 = min(j0 + 4, cb)
                    b0, b1_ = j0 * D, j1 * D
                    w = b1_ - b0
                    nj = j1 - j0
                    y0 = epool.tile([128, 4 * D], fp32, tag="eb0")
                    nc.vector.tensor_tensor(
                        y0[:, 0:w].rearrange("l (j c) -> l j c", c=D),
                        pre_buf[:, b0:b1_].rearrange("l (j c) -> l j c", c=D),
                        b2row[:].unsqueeze(1).broadcast_to([128, nj, D]),
                        ALU.add)
                    y1 = epool.tile([128, 4 * D], fp32, tag="eb1")
                    nc.vector.tensor_tensor(y1[:, 0:w], y0[:, 0:w],
                                            xres16[:, b0:b1_], ALU.add)
                    nc.sync.dma_start(
                        y_d.ap().rearrange("(j l) c -> l j c", l=128)
                        [:, j0:j1, :],
                        y1[:, 0:w].rearrange("l (j c) -> l j c", c=D))

            def finish_early():
                y_stub = spool.tile([128, D], fp32, tag="ystub")
                nc.vector.memset(y_stub[:], 0.0)
                nc.sync.dma_start(y_d.ap()[0:128, :], y_stub[:])

            WFREE = NCHUNK * D
            NB = 512

            # ================= layer 1 =================
            def x_slab(s):
                xs = slab.tile([D + 1, PCORE], fp16, tag="slab")
                nc.sync.dma_start(xs[:], xT_full_d.ap()[:, s * PCORE:(s + 1) * PCORE])
                return xs

            QMAX = max(cb - ca for (_, ca, cb) in qruns)

            def table2_build(q, s0, s1):
                """Table-2 rows for piece q, source slabs [s0, s1).  Paced two
                pieces behind the AllGather launches so h_tall[q] is ready."""
                _, qa, qb = qruns[q]
                qw = (qb - qa) * 128
                qn = qb - qa
                for s in range(s0, s1):
                    xs = slab.tile([D + 1, QMAX * 128], fp8, tag="slab2")
                    nc.sync.dma_start(xs[:, 0:qw],
                                      h_tall_q[q].ap()[s * (D + 1):(s + 1) * (D + 1), :])
                    tb = dpool.tile([128, max(QMAX, (NCHUNK + 2) // 3), TCOL],
                                    fp16, tag="tabs")
                    for j in range(qn):
                        ps = psum_p.tile([128, TCOL], fp32, tag="dps")
                        nc.tensor.matmul(ps[:], xs[:, j * 128:(j + 1) * 128],
                                         Wb2_tab8[:], start=True, stop=True)
                        if j % 2 == 0:
                            nc.scalar.activation(tb[:, j, :], ps[:], AF.Copy)
                        else:
                            nc.vector.tensor_copy(tb[:, j, :], ps[:])
                    r0 = (s * NCHUNK + qa) * 128
                    nc.sync.dma_start(
                        T_d[1].ap()[r0:r0 + qn * 128, 0:TCOL]
                        .rearrange("(b l) c -> l b c", l=128),
                        tb[:, 0:qn, :])

            import functools
            table_pass(0, x_slab)
            own_pass(0)
            load_ae(0)
            load_ae(1)
            prep_aeadst(0)
            done = stage == "dense1"
            if not done:
                # Layer-2 prep is interleaved into edge phase 1 as post-group
                # tasks: epilogue (h + own2/prep2) at each piece's end group,
                # collective launch one group later, table-2 builds two pieces
                # behind (split into slab halves across adjacent groups).
                tasks0 = {}
                post0 = []

                def _at(gi, fn):
                    if gi < NGRP:
                        tasks0.setdefault(gi, []).append(fn)
                    else:
                        post0.append(fn)

                P = functools.partial
                if stage != "edge1":
                    for q, (gi, ca, cb) in enumerate(qruns):
                        _at(gi, P(quarter_epilogue, q))
                        # Defer the launch one group so its sem wait doesn't
                        # stall the Pool queue mid-stream — except the last
                        # two pieces, where collective earliness wins.
                        _at(gi if q >= NQ - 2 else gi + 1,
                            P(launch_collective, q))
                    # Layer-2 dense prep + all table-2 builds go after edge
                    # phase 1: its DMA is saturated, while in the collective
                    # trough DVE (own/prep) and DMA (builds) are both free.
                    # Pieces 0..NQ-2 landed during edge 1, so the builds
                    # stream without stalls; build NQ-1 waits only the tiny
                    # last piece, which overlaps the earlier builds.
                    post0.append(P(own_pass, 1))
                    post0.append(P(prep_aeadst, 1))
                    for q in range(NQ - 1):
                        post0.append(P(table2_build, q, 0, NCORE))
                edge_phase(0, tasks0)
                for fn in post0:
                    fn()
                done = stage == "edge1"
            if done:
                finish_early()
            else:
                # ================= layer 2 =================
                table2_build(NQ - 1, 0, NCORE)
                if stage == "dense2":
                    finish_early()
                else:
                    tasks1 = {}
                    post1 = []
                    for q, (gi, ca, cb) in enumerate(qruns):
                        if gi + 1 < NGRP:
                            tasks1.setdefault(gi + 1, []).append(
                                functools.partial(y_quarter, q))
                        else:
                            post1.append(functools.partial(y_quarter, q))
                    edge_phase(1, tasks1)
                    for fn in post1:
                        fn()

    nc.compile()
    return nc


def _get_nc(cfg):
    import os
    stage = os.environ.get("KERNEL_STAGE", "full")
    key = (tuple(sorted(cfg.items())), stage)
    if key not in _BUILD_CACHE:
        _BUILD_CACHE[key] = _build(key[0], stage)
    return _BUILD_CACHE[key]


# ----------------------------------------------------------------------------
# Entry point
# ----------------------------------------------------------------------------

def kernel(**inputs):
    import sys
    if "/opt/trn_rl_repo" not in sys.path:
        sys.path.insert(0, "/opt/trn_rl_repo")
    from concourse.bass_utils import run_bass_kernel_spmd

    cfg, in_maps, perm = _prepare(**inputs)
    nc = _get_nc(cfg)
    res = run_bass_kernel_spmd(nc, in_maps, core_ids=list(range(NCORE)))
    kernel.last_results = res

    N, D = cfg["N"], cfg["D"]
    y = np.empty((N, D), dtype=np.float32)
    for c in range(NCORE):
        n = perm[c]
        valid = n >= 0
        y[n[valid]] = res.results[c]["y"][:valid.sum()]
    return y



# revision 32
# speedup vs baseline: 1.0742x; 1.0656x over previous
"""Trainium2 Bass kernel for a 2-layer edge-conditioned GAT (PyG GATConv style).

Strategy (8 NeuronCores, SPMD, node-parallel):
  - Nodes dealt to cores round-robin in degree order; each core owns softmax +
    aggregation for its nodes.  Per core, nodes are bucketed into 128-lane
    chunks; incoming edges form a padded [lane, slot] grid so per-edge ops are
    dense tile ops.
  - The per-layer gather table ([xl | 1 | a_src] per node, 66 fp16 cols in
    256B-strided rows) is computed REPLICATED on every core by a cheap dense
    matmul pass (layer 1 from the replicated x; layer 2 from an fp16
    AllGather of h^T).  No table AllGather.
  - Edge gathers use SWDGE dma_gather with int16 indices.  The >32768-row
    range is covered by NW=5 overlapping 32768-row windows; edges in window
    overlaps are assigned to windows so as to minimize the padded per-chunk
    grid widths (min-cost interval assignment, SPMD-uniform across cores).
  - Scores: e = lrelu(a_src[src] + a_dst[dst] + c*ea); the max-shift of the
    reference softmax is replaced by a constant shift (exact: softmax is
    shift invariant; scores are bounded).  The appended all-ones table column
    makes the softmax denominator fall out of the same fused multiply+reduce
    that aggregates features.
"""

import math

import numpy as np

NCORE = 8
ROW = 128          # fp16 elements per gather-table row (= 256B, SWDGE minimum)
TCOL = 66          # used table columns: [xl(0:64) | one | a_src]
B_MAX = 144        # max gather blocks (of 128 edges) per group
SUB_BLK = 8        # blocks per dma_gather call (1024 idxs = Q7 ucode scratch cap)
SCRATCH = 16384    # SWDGE ring carveout (1024 descriptors; the gather stream
                   # rate is identical down to this size, and it frees 16KB
                   # of SBUF per partition)
LIM = 32768        # int16 gather window (rows per window)
NW = 5             # gather windows
EXP_SHIFT = -8.0   # constant softmax shift
PAD_AE = -60000.0  # score for padded slots -> exp == 0

_BUILD_CACHE = {}


# ----------------------------------------------------------------------------
# Host-side preprocessing
# ----------------------------------------------------------------------------

def _pack(src_p, dst_core, dst_chunk, dst_lane, NCHUNK, S):
    """4-window grid packing.  Returns per-chunk per-window widths W [NCHUNK,NW],
    and per-edge (window, depth) assignments.  Widths are shared across cores
    (SPMD-uniform program)."""
    EE = len(src_p)
    # window-coverage interval [lo, hi] per edge (coverage is contiguous)
    lo = np.zeros(EE, np.int8)
    hi = np.zeros(EE, np.int8)
    cov = np.zeros((NW, EE), bool)
    for w in range(NW):
        cov[w] = (src_p >= S[w]) & (src_p < S[w] + LIM)
    lo = np.argmax(cov, axis=0).astype(np.int8)
    hi = (NW - 1 - np.argmax(cov[::-1], axis=0)).astype(np.int8)
    assert (cov[lo, np.arange(EE)] & cov[hi, np.arange(EE)]).all()

    gkey = (dst_core.astype(np.int64) * NCHUNK + dst_chunk) * 128 + dst_lane
    NKEY = NCORE * NCHUNK * 128

    # per-key demand per class (class = (lo, hi) pair); classes are few
    classes = sorted({(int(a), int(b)) for a, b in zip(lo, hi)})
    cidx = {c: i for i, c in enumerate(classes)}
    ecls = np.array([cidx[(int(a), int(b))] for a, b in zip(lo, hi)],
                    dtype=np.int8)
    NCLS = len(classes)
    dem_cls = np.zeros((NKEY, NCLS), np.int32)
    np.add.at(dem_cls, (gkey, ecls), 1)

    key_chunk = (np.arange(NKEY) // 128) % NCHUNK

    # interval-constraint DP for per-chunk widths (joint across cores):
    # c[k+1] = max_i (c[i] + dem[i,k]) where dem[i,k] = max over keys of
    # edges whose interval is within [i, k]
    W = np.zeros((NCHUNK, NW), np.int64)
    for j in range(NCHUNK):
        sel = key_chunk == j
        dj = dem_cls[sel]
        dem = {}
        for i in range(NW):
            for k in range(i, NW):
                csel = [cidx[c] for c in classes
                        if c[0] >= i and c[1] <= k]
                dem[(i, k)] = int(dj[:, csel].sum(axis=1).max()) if csel else 0
        c = [0] * (NW + 1)
        for k in range(NW):
            c[k + 1] = max([c[i] + dem[(i, k)] for i in range(k + 1)] + [c[k]])
        c[1] = max(c[1], 1)
        for k in range(1, NW):
            c[k + 1] = max(c[k + 1], c[k])
        W[j] = np.diff(np.array(c))
    assert (W[:, 0] >= 1).all()

    # per-key greedy assignment: for w in 0..NW-1 take classes by ascending hi
    remaining = dem_cls.astype(np.int64).copy()
    take = np.zeros((NKEY, NCLS, NW), np.int32)   # edges of class -> window
    order = sorted(range(NCLS), key=lambda ci: (classes[ci][1], classes[ci][0]))
    for w in range(NW):
        cap = W[key_chunk, w].copy()
        for ci in order:
            clo, chi = classes[ci]
            if not (clo <= w <= chi):
                continue
            if chi == w:
                t = remaining[:, ci].copy()   # must take all
            else:
                t = np.minimum(remaining[:, ci], cap)
            take[:, ci, w] = t
            cap -= t
            remaining[:, ci] -= t
        assert (cap >= 0).all(), f"window {w} overflow"
    assert (remaining == 0).all(), "assignment infeasible"

    # per-edge window: position within (key, class) decides the window
    eorder = np.lexsort((ecls, gkey))
    kc_sorted = gkey[eorder] * np.int64(NCLS) + ecls[eorder]
    starts = np.r_[0, np.flatnonzero(np.diff(kc_sorted)) + 1]
    counts = np.diff(np.r_[starts, EE])
    posin = np.arange(EE) - np.repeat(starts, counts)
    cum = np.cumsum(take, axis=2)               # [NKEY, NCLS, NW]
    pos_e = np.empty(EE, np.int64)
    pos_e[eorder] = posin
    cum_e = cum[gkey, ecls]                     # [EE, NW]
    win_e = (pos_e[:, None] >= cum_e).sum(axis=1).astype(np.int8)
    assert (win_e < NW).all()
    assert (cov[win_e, np.arange(EE)]).all()

    # depth within (key, window)
    kw = gkey * np.int64(NW) + win_e
    o2 = np.argsort(kw, kind="stable")
    kw_s = kw[o2]
    st2 = np.r_[0, np.flatnonzero(np.diff(kw_s)) + 1]
    cn2 = np.diff(np.r_[st2, EE])
    k_s = np.arange(EE) - np.repeat(st2, cn2)
    depth_e = np.empty(EE, np.int64)
    depth_e[o2] = k_s
    assert (depth_e < W[dst_chunk, win_e]).all()
    return W, win_e, depth_e


def _prepare(x, edge_index, edge_attr,
             W_res, b_res, alpha_mix,
             W1, att_src1, att_dst1, We1, att_e1, b1,
             W2, att_src2, att_dst2, We2, att_e2, b2):
    N, D = x.shape
    E = edge_index.shape[1]
    f32 = np.float32

    src = np.concatenate([edge_index[0], np.arange(N, dtype=np.int64)]).astype(np.int64)
    dst = np.concatenate([edge_index[1], np.arange(N, dtype=np.int64)]).astype(np.int64)
    ea = np.concatenate([edge_attr[:, 0].astype(f32),
                         np.full(N, edge_attr.astype(f32).mean(), dtype=f32)])
    EE = E + N

    deg = np.bincount(dst, minlength=N)

    PCORE = int(math.ceil(N / NCORE / 128) * 128)
    NCHUNK = PCORE // 128
    TROWS = NCORE * PCORE
    B0 = TROWS - LIM
    S = tuple(int(round(i * B0 / (NW - 1))) for i in range(NW))

    # Deal nodes to cores round-robin in degree order (balances edge counts);
    # within a core sort by degree so chunk members have similar degree.
    order = np.argsort(deg, kind="stable")
    rank = np.empty(N, dtype=np.int64)
    rank[order] = np.arange(N)
    core_of = (rank % NCORE).astype(np.int32)
    pos_in_core = np.full(N, -1, dtype=np.int64)
    perm = np.full((NCORE, PCORE), -1, dtype=np.int64)
    for c in range(NCORE):
        nodes = np.where(core_of == c)[0]
        nodes = nodes[np.argsort(deg[nodes], kind="stable")]
        pos_in_core[nodes] = np.arange(len(nodes))
        perm[c, :len(nodes)] = nodes
    p_id = core_of.astype(np.int64) * PCORE + pos_in_core

    src_p = p_id[src]
    dst_core = core_of[dst]
    dst_chunk = pos_in_core[dst] // 128
    dst_lane = pos_in_core[dst] % 128

    W, win_e, depth_e = _pack(src_p, dst_core, dst_chunk, dst_lane, NCHUNK, S)

    # Greedy group packing: chunks -> groups with <= B_MAX blocks each.
    # Group packing with a ramp: small first groups so the vector engine
    # starts while later gathers stream, and a small final group so the
    # exposed tail compute is short.
    wsum = W.sum(axis=1)
    groups = []
    gs, acc = 0, 0
    for j in range(NCHUNK):
        b = int(wsum[j])
        cap = (B_MAX // 3 if len(groups) == 0 else
               2 * B_MAX // 3 if len(groups) == 1 else B_MAX)
        if acc + b > cap and j > gs:
            groups.append((gs, j))
            gs, acc = j, 0
        acc += b
    groups.append((gs, NCHUNK))
    # Down-ramp the tail: the final group's gather + reduce + epilogue +
    # AllGather piece is the exposed critical path into edge phase 2, so
    # split trailing groups until the last one is small.
    while True:
        a, b = groups[-1]
        if b - a < 2 or wsum[a:b].sum() <= B_MAX // 3:
            break
        mid = a + (b - a) * 2 // 3
        if mid == a:
            break
        groups[-1:] = [(a, mid), (mid, b)]

    # Block layout per group: [win0 blocks of its chunks | win1 | win2 | win3]
    base = np.zeros((NCHUNK, NW), np.int64)
    group_info = []   # (blk0, (nblk per window), (ca, cb))
    bpos = 0
    for (a, b) in groups:
        blk0 = bpos
        nblk = []
        for w in range(NW):
            n0 = bpos
            for j in range(a, b):
                base[j, w] = bpos
                bpos += int(W[j, w])
            nblk.append(bpos - n0)
        group_info.append((blk0, tuple(nblk), (a, b)))
    B_TOT = bpos
    NSLOT = B_TOT * 128

    blk_e = base[dst_chunk, win_e] + depth_e
    slot_e = blk_e * 128 + dst_lane
    idx_val = (src_p - np.asarray(S, dtype=np.int64)[win_e])
    assert (idx_val >= 0).all() and (idx_val < LIM).all()
    idx_val = idx_val.astype(np.int16)

    c1 = float(np.dot(We1[0].astype(f32), att_e1.astype(f32)))
    c2 = float(np.dot(We2[0].astype(f32), att_e2.astype(f32)))

    idx_imgs, ae1_imgs, ae2_imgs = [], [], []
    for c in range(NCORE):
        m = dst_core == c
        sl = slot_e[m]
        grid_idx = np.zeros(NSLOT, dtype=np.int16)
        grid_idx[sl] = idx_val[m]
        g1 = np.full(NSLOT, PAD_AE, dtype=f32)
        g1[sl] = c1 * ea[m]
        g2 = np.full(NSLOT, PAD_AE, dtype=f32)
        g2[sl] = c2 * ea[m]
        img16 = grid_idx.reshape(-1, 16).T
        # dma_gather on queue 0 reads 32 idx channels; 2x-replicated 16-wrap
        idx_imgs.append(np.tile(img16, (2, 1)).copy())
        ae1_imgs.append(np.ascontiguousarray(g1.reshape(B_TOT, 128).T.astype(np.float16)))
        ae2_imgs.append(np.ascontiguousarray(g2.reshape(B_TOT, 128).T.astype(np.float16)))

    # Give fully-padded lanes (node-count padding) one live slot so s > 0.
    lane_has = np.zeros((NCORE, PCORE), dtype=bool)
    lane_has[dst_core, pos_in_core[dst]] = True
    for c in range(NCORE):
        for j in range(NCHUNK):
            dead = np.where(~lane_has[c, j * 128:(j + 1) * 128])[0]
            if len(dead):
                ae1_imgs[c][dead, base[j, 0]] = 0.0
                ae2_imgs[c][dead, base[j, 0]] = 0.0

    # Weights.  Residual Linear folded into layer 1; biases folded via the
    # ones row of the lhsT (layer 1) / zero-bias (layer 2).
    W_res = W_res.astype(f32)
    b_res = b_res.astype(f32)
    W1 = W1.astype(f32)
    W2 = W2.astype(f32)
    alpha = float(alpha_mix)
    W1e = W_res @ W1
    b1e = b_res @ W1

    # Table pass rhs, layer 1: cols [xl(0:64) | one | a_src], lhsT rows [x; 1]
    Wb1_tab = np.zeros((D + 1, TCOL), dtype=f32)
    Wb1_tab[:D, 0:D] = W1e
    Wb1_tab[D, 0:D] = b1e
    Wb1_tab[D, D] = 1.0
    Wb1_tab[:D, D + 1] = W1e @ att_src1.astype(f32)
    Wb1_tab[D, D + 1] = float(b1e @ att_src1.astype(f32))
    # Own pass rhs, layer 1: cols [a_dst | alpha*xres(0:64)]
    Wb1_own = np.zeros((D + 1, 1 + D), dtype=f32)
    Wb1_own[:D, 0] = W1e @ att_dst1.astype(f32)
    Wb1_own[D, 0] = float(b1e @ att_dst1.astype(f32))
    Wb1_own[:D, 1:] = W_res * alpha
    Wb1_own[D, 1:] = b_res * alpha
    # Table pass rhs, layer 2 (lhsT rows [h; 1])
    Wb2_tab = np.zeros((D + 1, TCOL), dtype=f32)
    Wb2_tab[:D, 0:D] = W2
    Wb2_tab[D, D] = 1.0
    Wb2_tab[:D, D + 1] = W2 @ att_src2.astype(f32)
    # Own pass rhs, layer 2: col [a_dst2]
    Wb2_own = np.zeros((D + 1, 1), dtype=f32)
    Wb2_own[:D, 0] = W2 @ att_dst2.astype(f32)

    # Dense inputs: x^T in p_id order with ones row, fp16.
    xT_full = np.zeros((D + 1, TROWS), dtype=np.float16)
    valid_all = perm.reshape(-1) >= 0
    cols = np.arange(TROWS)[valid_all]
    xT_full[:D, cols] = x[perm.reshape(-1)[valid_all]].astype(np.float16).T
    xT_full[D, :] = 1.0

    WMAXW = int(W.max())
    WMAXC = int(W.sum(axis=1).max())

    cfg = dict(
        N=N, D=D, PCORE=PCORE, NCHUNK=NCHUNK, TROWS=TROWS,
        B_TOT=B_TOT, NSLOT=NSLOT, WMAXW=WMAXW, WMAXC=WMAXC, S=S,
        W=tuple(tuple(int(v) for v in row) for row in W),
        base=tuple(tuple(int(v) for v in row) for row in base),
        groups=tuple((int(b0), tuple(int(n) for n in nblk), (int(a), int(b)))
                     for (b0, nblk, (a, b)) in group_info),
    )

    in_maps = []
    for c in range(NCORE):
        in_maps.append(dict(
            xT_full=xT_full,
            xT_own=np.ascontiguousarray(xT_full[:, c * PCORE:(c + 1) * PCORE]),
            idx_img=idx_imgs[c],
            ae1=ae1_imgs[c],
            ae2=ae2_imgs[c],
            Wb1_tab=Wb1_tab.astype(np.float16),
            Wb1_own=Wb1_own.astype(np.float16),
            Wb2_tab=Wb2_tab.astype(np.float16),
            Wb2_own=Wb2_own.astype(np.float16),
            b1row=np.tile(b1.astype(f32).reshape(1, D), (128, 1)),
            b2row=np.tile(b2.astype(f32).reshape(1, D), (128, 1)),
            ones_row=np.ones((1, PCORE), dtype=np.float16),
            ident=np.eye(128, dtype=np.float16),
        ))
    return cfg, in_maps, perm


# ----------------------------------------------------------------------------
# Device program
# ----------------------------------------------------------------------------

def _build(cfg_key, stage='full'):
    import contextlib

    import concourse.bass as bass
    import concourse.tile as tile
    import concourse.mybir as mybir
    from concourse import bacc
    from concourse.library_config import mlp

    cfg = dict(cfg_key)
    D = cfg["D"]
    PCORE, NCHUNK, TROWS = cfg["PCORE"], cfg["NCHUNK"], cfg["TROWS"]
    B_TOT, NSLOT, WMAXW = cfg["B_TOT"], cfg["NSLOT"], cfg["WMAXW"]
    WMAXC = cfg["WMAXC"]
    S = cfg["S"]
    W = cfg["W"]
    base = cfg["base"]
    groups = cfg["groups"]

    fp16 = mybir.dt.float16
    fp32 = mybir.dt.float32
    i16 = mybir.dt.int16
    AF = mybir.ActivationFunctionType
    ALU = mybir.AluOpType

    nc = bacc.Bacc("TRN2", target_bir_lowering=False, debug=False,
                   num_devices=NCORE, dynamic_dma_scratch_size=SCRATCH)

    xT_full_d = nc.dram_tensor("xT_full", [D + 1, TROWS], fp16, kind="ExternalInput")
    xT_own_d = nc.dram_tensor("xT_own", [D + 1, PCORE], fp16, kind="ExternalInput")
    idx_img = nc.dram_tensor("idx_img", [32, NSLOT // 16], i16, kind="ExternalInput")
    ae1_d = nc.dram_tensor("ae1", [128, B_TOT], fp16, kind="ExternalInput")
    ae2_d = nc.dram_tensor("ae2", [128, B_TOT], fp16, kind="ExternalInput")
    Wb1_tab_d = nc.dram_tensor("Wb1_tab", [D + 1, TCOL], fp16, kind="ExternalInput")
    Wb1_own_d = nc.dram_tensor("Wb1_own", [D + 1, 1 + D], fp16, kind="ExternalInput")
    Wb2_tab_d = nc.dram_tensor("Wb2_tab", [D + 1, TCOL], fp16, kind="ExternalInput")
    Wb2_own_d = nc.dram_tensor("Wb2_own", [D + 1, 1], fp16, kind="ExternalInput")
    b1row_d = nc.dram_tensor("b1row", [128, D], fp32, kind="ExternalInput")
    b2row_d = nc.dram_tensor("b2row", [128, D], fp32, kind="ExternalInput")
    ones_d = nc.dram_tensor("ones_row", [1, PCORE], fp16, kind="ExternalInput")
    ident_d = nc.dram_tensor("ident", [128, 128], fp16, kind="ExternalInput")
    y_d = nc.dram_tensor("y", [PCORE, D], fp32, kind="ExternalOutput")

    T_d = [nc.dram_tensor(f"T{l}", [TROWS, ROW], fp16) for l in range(2)]

    # Piece split of the group list: the h AllGather is pipelined in NQ
    # column pieces, each issued as soon as its chunks' epilogue is done so
    # the collective overlaps the remaining edge-phase groups.
    NGRP = len(groups)
    # ~2 groups per piece so the serial collective pipe starts early and
    # never stacks; the last piece is the (small) final group alone, since
    # its latency is what edge phase 2 waits on.
    qbound = list(range(2, NGRP - 1, 2))
    if NGRP - 1 not in qbound:
        qbound.append(NGRP - 1)
    qbound.append(NGRP)
    NQ = len(qbound)
    qruns = []
    g0 = 0
    for q in range(NQ):
        g1 = qbound[q]
        ca = groups[g0][2][0]
        cb = groups[g1 - 1][2][1]
        qruns.append((g1 - 1, ca, cb))
        g0 = g1
    # h travels fp8: per-node a_dst error cancels in the segment softmax
    # (constant shift per destination), and the xl2/a_src2 error is averaged
    # over D=64 terms.  Rows [h(64) | ones] so the table-2 matmul keeps its
    # denominator column without a separate ones load.
    fp8 = mybir.dt.float8e4
    h_tin_q = [nc.dram_tensor(f"h_tin{q}", [D + 1, (cb - ca) * 128], fp8)
               for q, (_, ca, cb) in enumerate(qruns)]
    h_tall_q = [nc.dram_tensor(f"h_tall{q}", [NCORE * (D + 1), (cb - ca) * 128],
                               fp8, addr_space="Shared")
                for q, (_, ca, cb) in enumerate(qruns)]

    nc.gpsimd.load_library(mlp)
    rg = [list(range(NCORE))]

    with tile.TileContext(nc) as tc:
        with contextlib.ExitStack() as ctx:
            resident = ctx.enter_context(tc.tile_pool(name="resident", bufs=1))
            slab = ctx.enter_context(tc.tile_pool(name="slab", bufs=2))
            gpool = ctx.enter_context(tc.tile_pool(name="gather", bufs=2))
            ppool = ctx.enter_context(tc.tile_pool(name="prod", bufs=2))
            spool = ctx.enter_context(tc.tile_pool(name="small", bufs=3))
            epool = ctx.enter_context(tc.tile_pool(name="epil", bufs=2))
            dpool = ctx.enter_context(tc.tile_pool(name="dense", bufs=3))
            ipool = ctx.enter_context(tc.tile_pool(name="idx", bufs=3))
            psum_p = ctx.enter_context(tc.tile_pool(name="ps", bufs=6, space="PSUM"))
            psum_t = ctx.enter_context(tc.tile_pool(name="pst", bufs=2, space="PSUM"))

            # ---------------- resident loads ----------------
            Wb1_tab_sb = resident.tile([D + 1, TCOL], fp16)
            nc.sync.dma_start(Wb1_tab_sb[:], Wb1_tab_d.ap())
            Wb1_own_sb = resident.tile([D + 1, 1 + D], fp16)
            nc.sync.dma_start(Wb1_own_sb[:], Wb1_own_d.ap())
            Wb2_tab_sb = resident.tile([D + 1, TCOL], fp16)
            nc.sync.dma_start(Wb2_tab_sb[:], Wb2_tab_d.ap())
            Wb2_own_sb = resident.tile([D + 1, 1], fp16)
            nc.sync.dma_start(Wb2_own_sb[:], Wb2_own_d.ap())
            b1row = resident.tile([128, D], fp32)
            nc.sync.dma_start(b1row[:], b1row_d.ap())
            b2row = resident.tile([128, D], fp32)
            nc.sync.dma_start(b2row[:], b2row_d.ap())
            ident = resident.tile([128, 128], fp16)
            nc.sync.dma_start(ident[:], ident_d.ap())
            expshift = resident.tile([128, 1], fp32)
            nc.vector.memset(expshift[:], EXP_SHIFT)

            h_T = resident.tile([D + 1, PCORE], fp8)
            nc.vector.memset(h_T[D:D + 1, :], 1.0)
            Wb2_tab8 = resident.tile([D + 1, TCOL], fp8)
            nc.vector.tensor_copy(Wb2_tab8[:], Wb2_tab_sb[:])
            Wb2_own8 = resident.tile([D + 1, 1], fp8)
            nc.vector.tensor_copy(Wb2_own8[:], Wb2_own_sb[:])

            ae_sb = [resident.tile([128, B_TOT], fp16, name=f"ae_sb{l}")
                     for l in range(2)]
            aeadst = [resident.tile([128, B_TOT], fp16, name=f"aeadst{l}")
                      for l in range(2)]
            xres16 = resident.tile([128, NCHUNK * D], fp16)
            h_sb = resident.tile([128, NCHUNK * D], fp16)
            adst = [resident.tile([128, NCHUNK], fp32, name=f"adst{l}")
                    for l in range(2)]
            pre_buf = resident.tile([128, NCHUNK * D], fp32)

            def table_pass(layer, lhsT_src):
                """Write the full gather table T[layer] from dense matmuls.
                lhsT_src(s) -> loads slab s ([D+1, PCORE]) and returns tile.
                One batched DMA write per slab (per-chunk writes serialize on
                the HWDGE fixed overhead); PSUM->SBUF staging alternates
                between the Activation and Vector engines."""
                half = (NCHUNK + 2) // 3
                for s in range(NCORE):
                    xs = lhsT_src(s)
                    for j0 in range(0, NCHUNK, half):
                        j1 = min(j0 + half, NCHUNK)
                        tb = dpool.tile([128, half, TCOL], fp16, tag="tabs")
                        for j in range(j0, j1):
                            ps = psum_p.tile([128, TCOL], fp32, tag="dps")
                            nc.tensor.matmul(ps[:], xs[:, j * 128:(j + 1) * 128],
                                             (Wb1_tab_sb if layer == 0 else Wb2_tab_sb)[:],
                                             start=True, stop=True)
                            if j % 2 == 0:
                                nc.scalar.activation(tb[:, j - j0, :], ps[:], AF.Copy)
                            else:
                                nc.vector.tensor_copy(tb[:, j - j0, :], ps[:])
                        r0 = (s * NCHUNK + j0) * 128
                        nc.sync.dma_start(
                            T_d[layer].ap()[r0:r0 + (j1 - j0) * 128, 0:TCOL]
                            .rearrange("(b l) c -> l b c", l=128),
                            tb[:, 0:j1 - j0, :])

            def own_pass(layer, ja=0, jb=NCHUNK, lhsT=None):
                if layer == 0:
                    xo = slab.tile([D + 1, PCORE], fp16, tag="slab")
                    nc.sync.dma_start(xo[:], xT_own_d.ap())
                    lhsT = xo
                elif lhsT is None:
                    lhsT = h_T
                ncols = (1 + D) if layer == 0 else 1
                W_own = Wb1_own_sb if layer == 0 else Wb2_own8
                for j in range(ja, jb):
                    ps = psum_p.tile([128, TCOL], fp32, tag="dps")
                    nc.tensor.matmul(ps[:, 0:ncols], lhsT[:, j * 128:(j + 1) * 128],
                                     W_own[:], start=True, stop=True)
                    nc.vector.tensor_copy(adst[layer][:, j:j + 1], ps[:, 0:1])
                    if layer == 0:
                        nc.scalar.activation(
                            xres16[:, j * D:(j + 1) * D], ps[:, 1:1 + D], AF.Copy)

            def quarter_epilogue(q):
                """h = elu(pre + b1) for quarter q's chunks, transpose into
                h_T, write h_tin[q] and kick its AllGather.  Emitted mid
                edge-phase-1 so the collective overlaps later groups."""
                _, ca, cb = qruns[q]
                for j0 in range(ca, cb, 4):
                    j1 = min(j0 + 4, cb)
                    b0, b1_ = j0 * D, j1 * D
                    w = b1_ - b0
                    nj = j1 - j0
                    t0 = epool.tile([128, 4 * D], fp32, tag="eb0")
                    nc.vector.tensor_tensor(
                        t0[:, 0:w].rearrange("l (j c) -> l j c", c=D),
                        pre_buf[:, b0:b1_].rearrange("l (j c) -> l j c", c=D),
                        b1row[:].unsqueeze(1).broadcast_to([128, nj, D]), ALU.add)
                    mneg = epool.tile([128, 4 * D], fp32, tag="eb1")
                    nc.vector.tensor_scalar_min(mneg[:, 0:w], t0[:, 0:w], 0.0)
                    eneg = epool.tile([128, 4 * D], fp32, tag="eb2")
                    nc.scalar.activation(eneg[:, 0:w], mneg[:, 0:w], AF.Exp)
                    ppos = epool.tile([128, 4 * D], fp32, tag="eb1b")
                    nc.vector.tensor_scalar_max(ppos[:, 0:w], t0[:, 0:w], 0.0)
                    nc.vector.scalar_tensor_tensor(
                        h_sb[:, b0:b1_], eneg[:, 0:w], -1.0, ppos[:, 0:w],
                        ALU.add, ALU.add)
                for j in range(ca, cb):
                    pt = psum_t.tile([D, 128], fp16, tag="pt")
                    nc.tensor.transpose(pt[:], h_sb[:, j * D:(j + 1) * D], ident[:])
                    nc.vector.tensor_copy(h_T[0:D, j * 128:(j + 1) * 128], pt[:])

            def launch_collective(q):
                """h_tin write + AllGather for piece q.  Emitted one group
                after the epilogue compute so its sem waits don't stall the
                in-order SP/Pool queues mid-stream."""
                _, ca, cb = qruns[q]
                nc.sync.dma_start(h_tin_q[q].ap(),
                                  h_T[:, ca * 128:cb * 128])
                nc.gpsimd.collective_compute(
                    "AllGather", ALU.bypass, replica_groups=rg,
                    ins=[h_tin_q[q].ap().opt()], outs=[h_tall_q[q].ap().opt()])

            def load_ae(layer):
                nc.sync.dma_start(ae_sb[layer][:],
                                  (ae1_d if layer == 0 else ae2_d).ap())

            def prep_aeadst(layer, ja=0, jb=NCHUNK):
                """ae + a_dst per slot for chunks [ja, jb)."""
                for j in range(ja, jb):
                    for w in range(NW):
                        if W[j][w]:
                            b0 = base[j][w]
                            nc.vector.tensor_scalar_add(
                                aeadst[layer][:, b0:b0 + W[j][w]],
                                ae_sb[layer][:, b0:b0 + W[j][w]],
                                adst[layer][:, j:j + 1])

            def edge_phase(layer, tasks=None):
                table = T_d[layer]
                for gi, (blk0, nblk, (ca, cb)) in enumerate(groups):
                    bg = sum(nblk)
                    G = gpool.tile([128, B_MAX, ROW], fp16, tag="G")
                    it = ipool.tile([32, B_MAX * 8], i16, tag="it")
                    nc.sync.dma_start(it[:, 0:bg * 8],
                                      idx_img.ap()[:, blk0 * 8:(blk0 + bg) * 8])
                    off = 0
                    for w in range(NW):
                        for s0 in range(0, nblk[w], SUB_BLK):
                            nb = min(SUB_BLK, nblk[w] - s0)
                            o = off + s0
                            nc.gpsimd.dma_gather(
                                G[:, o:o + nb, :],
                                table.ap()[S[w]:S[w] + LIM, :],
                                it[:, o * 8:(o + nb) * 8], nb * 128, nb * 128, ROW)
                        off += nblk[w]
                    u = spool.tile([128, B_MAX], fp32, tag="u")
                    nc.vector.tensor_tensor(
                        u[:, 0:bg], G[:, 0:bg, D + 1:D + 2].squeeze(2),
                        aeadst[layer][:, blk0:blk0 + bg], ALU.add)
                    t = spool.tile([128, B_MAX], fp32, tag="t")
                    nc.vector.scalar_tensor_tensor(
                        t[:, 0:bg], u[:, 0:bg], 0.2, u[:, 0:bg],
                        ALU.mult, ALU.max)
                    ex = spool.tile([128, B_MAX], fp16, tag="ex")
                    nc.scalar.activation(ex[:, 0:bg], t[:, 0:bg], AF.Exp,
                                         bias=expshift[:])
                    for j in range(ca, cb):
                        # P holds the chunk's windows back to back so one
                        # reduce covers the whole neighborhood.
                        P = ppool.tile([128, WMAXC, D + 1], fp16, tag="P")
                        po = 0
                        for w in range(NW):
                            dd = W[j][w]
                            if not dd:
                                continue
                            r0 = base[j][w] - blk0
                            nc.vector.tensor_tensor(
                                P[:, po:po + dd, :], G[:, r0:r0 + dd, 0:D + 1],
                                ex[:, r0:r0 + dd].unsqueeze(2)
                                .broadcast_to([128, dd, D + 1]),
                                ALU.mult)
                            po += dd
                        acc = spool.tile([128, D + 1], fp32, tag="red")
                        nc.vector.tensor_reduce(
                            acc[:], P[:, 0:po, :].transpose([0, 2, 1]),
                            axis=mybir.AxisListType.X, op=ALU.add)
                        rs = spool.tile([128, 1], fp32, tag="rs")
                        nc.vector.reciprocal(rs[:], acc[:, D:D + 1])
                        nc.vector.tensor_scalar_mul(
                            pre_buf[:, j * D:(j + 1) * D], acc[:, 0:D], rs[:])
                    if tasks:
                        for fn in tasks.get(gi, ()):
                            fn()

            def y_quarter(q):
                """y = pre + b2 + alpha*x_res for quarter q's chunks, written
                out as soon as they are reduced (overlaps later edge-2 groups)."""
                _, ca, cb = qruns[q]
                for j0 in range(ca, cb, 4):
                    j1 = min(j0 + 4, cb)
                    b0, b1_ = j0 * D, j1 * D
                    w = b1_ - b0
                    nj = j1 - j0
                    y0 = epool.tile([128, 4 * D], fp32, tag="eb0")
                    nc.vector.tensor_tensor(
                        y0[:, 0:w].rearrange("l (j c) -> l j c", c=D),
                        pre_buf[:, b0:b1_].rearrange("l (j c) -> l j c", c=D),
                        b2row[:].unsqueeze(1).broadcast_to([128, nj, D]),
                        ALU.add)
                    y1 = epool.tile([128, 4 * D], fp32, tag="eb1")
                    nc.vector.tensor_tensor(y1[:, 0:w], y0[:, 0:w],
                                            xres16[:, b0:b1_], ALU.add)
                    nc.sync.dma_start(
                        y_d.ap().rearrange("(j l) c -> l j c", l=128)
                        [:, j0:j1, :],
                        y1[:, 0:w].rearrange("l (j c) -> l j c", c=D))

            def finish_early():
                y_stub = spool.tile([128, D], fp32, tag="ystub")
                nc.vector.memset(y_stub[:], 0.0)
                nc.sync.dma_start(y_d.ap()[0:128, :], y_stub[:])

            WFREE = NCHUNK * D
            NB = 512

            # ================= layer 1 =================
            def x_slab(s):
                xs = slab.tile([D + 1, PCORE], fp16, tag="slab")
                nc.sync.dma_start(xs[:], xT_full_d.ap()[:, s * PCORE:(s + 1) * PCORE])
                return xs

            QMAX = max(cb - ca for (_, ca, cb) in qruns)

            def table2_build(q, s0, s1):
                """Table-2 rows for piece q, source slabs [s0, s1).  Paced two
                pieces behind the AllGather launches so h_tall[q] is ready."""
                _, qa, qb = qruns[q]
                qw = (qb - qa) * 128
                qn = qb - qa
                for s in range(s0, s1):
                    xs = slab.tile([D + 1, QMAX * 128], fp8, tag="slab2")
                    nc.sync.dma_start(xs[:, 0:qw],
                                      h_tall_q[q].ap()[s * (D + 1):(s + 1) * (D + 1), :])
                    tb = dpool.tile([128, max(QMAX, (NCHUNK + 2) // 3), TCOL],
                                    fp16, tag="tabs")
                    for j in range(qn):
                        ps = psum_p.tile([128, TCOL], fp32, tag="dps")
                        nc.tensor.matmul(ps[:], xs[:, j * 128:(j + 1) * 128],
                                         Wb2_tab8[:], start=True, stop=True)
                        if j % 2 == 0:
                            nc.scalar.activation(tb[:, j, :], ps[:], AF.Copy)
                        else:
                            nc.vector.tensor_copy(tb[:, j, :], ps[:])
                    r0 = (s * NCHUNK + qa) * 128
                    nc.sync.dma_start(
                        T_d[1].ap()[r0:r0 + qn * 128, 0:TCOL]
                        .rearrange("(b l) c -> l b c", l=128),
                        tb[:, 0:qn, :])

            import functools
            table_pass(0, x_slab)
            own_pass(0)
            load_ae(0)
            load_ae(1)
            # aeadst prep is ~25us of small DVE ops; emitted all upfront it
            # delays the first groups' score compute (and thus G-tile reuse,
            # stalling the gather stream).  Prep just the first groups now,
            # the rest two groups ahead of use.
            prep_aeadst(0, groups[0][2][0], groups[min(2, NGRP - 1)][2][1])
            done = stage == "dense1"
            if not done:
                # Layer-2 prep is interleaved into edge phase 1 as post-group
                # tasks: epilogue (h + own2/prep2) at each piece's end group,
                # collective launch one group later, table-2 builds two pieces
                # behind (split into slab halves across adjacent groups).
                tasks0 = {}
                post0 = []

                def _at(gi, fn):
                    if gi < NGRP:
                        tasks0.setdefault(gi, []).append(fn)
                    else:
                        post0.append(fn)

                P = functools.partial
                # rest of layer-1 aeadst prep, two groups ahead of use
                for g in range(3, NGRP):
                    _at(g - 2, P(prep_aeadst, 0,
                                 groups[g][2][0], groups[g][2][1]))
                if stage != "edge1":
                    for q, (gi, ca, cb) in enumerate(qruns):
                        _at(gi, P(quarter_epilogue, q))
                        # Defer the launch one group so its sem wait doesn't
                        # stall the Pool queue mid-stream — except the last
                        # two pieces, where collective earliness wins.
                        _at(gi if q >= NQ - 2 else gi + 1,
                            P(launch_collective, q))
                        # Layer-2 dense prep (own + aeadst) two groups after
                        # the epilogue that produced this piece's h columns:
                        # DVE has mid-stream slack, the trough doesn't.
                        _at(min(gi + 2, NGRP - 1), P(own_pass, 1, ca, cb))
                        _at(min(gi + 2, NGRP - 1), P(prep_aeadst, 1, ca, cb))
                    # Table-2 builds go after edge phase 1 (its DMA pipe is
                    # saturated; the trough's is free).  Slab-phased order:
                    # rows < 37632 (slabs 0-5) are written first across all
                    # pieces, which unblocks edge-2's window-0/1 gathers
                    # (range-based DRAM deps) while slabs 6-7 still build.
                    for q in range(NQ):
                        post0.append(P(table2_build, q, 0, 6))
                    for q in range(NQ):
                        post0.append(P(table2_build, q, 6, 7))
                    for q in range(NQ):
                        post0.append(P(table2_build, q, 7, NCORE))
                edge_phase(0, tasks0)
                for fn in post0:
                    fn()
                done = stage == "edge1"
            if done:
                finish_early()
            else:
                # ================= layer 2 =================
                if stage == "dense2":
                    finish_early()
                else:
                    tasks1 = {}
                    post1 = []
                    for q, (gi, ca, cb) in enumerate(qruns):
                        if gi + 1 < NGRP:
                            tasks1.setdefault(gi + 1, []).append(
                                functools.partial(y_quarter, q))
                        else:
                            post1.append(functools.partial(y_quarter, q))
                    edge_phase(1, tasks1)
                    for fn in post1:
                        fn()

    nc.compile()
    return nc


def _get_nc(cfg):
    import os
    stage = os.environ.get("KERNEL_STAGE", "full")
    key = (tuple(sorted(cfg.items())), stage)
    if key not in _BUILD_CACHE:
        _BUILD_CACHE[key] = _build(key[0], stage)
    return _BUILD_CACHE[key]


# ----------------------------------------------------------------------------
# Entry point
# ----------------------------------------------------------------------------

def kernel(**inputs):
    import sys
    if "/opt/trn_rl_repo" not in sys.path:
        sys.path.insert(0, "/opt/trn_rl_repo")
    from concourse.bass_utils import run_bass_kernel_spmd

    cfg, in_maps, perm = _prepare(**inputs)
    nc = _get_nc(cfg)
    res = run_bass_kernel_spmd(nc, in_maps, core_ids=list(range(NCORE)))
    kernel.last_results = res

    N, D = cfg["N"], cfg["D"]
    y = np.empty((N, D), dtype=np.float32)
    for c in range(NCORE):
        n = perm[c]
        valid = n >= 0
        y[n[valid]] = res.results[c]["y"][:valid.sum()]
    return y



# revision 34
# speedup vs baseline: 1.1009x; 1.0249x over previous
"""Trainium2 Bass kernel for a 2-layer edge-conditioned GAT (PyG GATConv style).

Strategy (8 NeuronCores, SPMD, node-parallel):
  - Nodes dealt to cores round-robin in degree order; each core owns softmax +
    aggregation for its nodes.  Per core, nodes are bucketed into 128-lane
    chunks; incoming edges form a padded [lane, slot] grid so per-edge ops are
    dense tile ops.
  - The per-layer gather table ([xl | 1 | a_src] per node, 66 fp16 cols in
    256B-strided rows) is computed REPLICATED on every core by a cheap dense
    matmul pass (layer 1 from the replicated x; layer 2 from an fp16
    AllGather of h^T).  No table AllGather.
  - Edge gathers use SWDGE dma_gather with int16 indices.  The >32768-row
    range is covered by NW=5 overlapping 32768-row windows; edges in window
    overlaps are assigned to windows so as to minimize the padded per-chunk
    grid widths (min-cost interval assignment, SPMD-uniform across cores).
  - Scores: e = lrelu(a_src[src] + a_dst[dst] + c*ea); the max-shift of the
    reference softmax is replaced by a constant shift (exact: softmax is
    shift invariant; scores are bounded).  The appended all-ones table column
    makes the softmax denominator fall out of the same fused multiply+reduce
    that aggregates features.
"""

import math

import numpy as np

NCORE = 8
ROW = 128          # fp16 elements per gather-table row (= 256B, SWDGE minimum)
TCOL = 66          # used table columns: [xl(0:64) | one | a_src]
B_MAX = 144        # max gather blocks (of 128 edges) per group
SUB_BLK = 8        # blocks per dma_gather call (1024 idxs = Q7 ucode scratch cap)
SCRATCH = 16384    # SWDGE ring carveout (1024 descriptors; the gather stream
                   # rate is identical down to this size, and it frees 16KB
                   # of SBUF per partition)
LIM = 32768        # int16 gather window (rows per window)
NW = 5             # gather windows
EXP_SHIFT = -8.0   # constant softmax shift
PAD_AE = -60000.0  # score for padded slots -> exp == 0

_BUILD_CACHE = {}


# ----------------------------------------------------------------------------
# Host-side preprocessing
# ----------------------------------------------------------------------------

def _pack(src_p, dst_core, dst_chunk, dst_lane, NCHUNK, S):
    """4-window grid packing.  Returns per-chunk per-window widths W [NCHUNK,NW],
    and per-edge (window, depth) assignments.  Widths are shared across cores
    (SPMD-uniform program)."""
    EE = len(src_p)
    # window-coverage interval [lo, hi] per edge (coverage is contiguous)
    lo = np.zeros(EE, np.int8)
    hi = np.zeros(EE, np.int8)
    cov = np.zeros((NW, EE), bool)
    for w in range(NW):
        cov[w] = (src_p >= S[w]) & (src_p < S[w] + LIM)
    lo = np.argmax(cov, axis=0).astype(np.int8)
    hi = (NW - 1 - np.argmax(cov[::-1], axis=0)).astype(np.int8)
    assert (cov[lo, np.arange(EE)] & cov[hi, np.arange(EE)]).all()

    gkey = (dst_core.astype(np.int64) * NCHUNK + dst_chunk) * 128 + dst_lane
    NKEY = NCORE * NCHUNK * 128

    # per-key demand per class (class = (lo, hi) pair); classes are few
    classes = sorted({(int(a), int(b)) for a, b in zip(lo, hi)})
    cidx = {c: i for i, c in enumerate(classes)}
    ecls = np.array([cidx[(int(a), int(b))] for a, b in zip(lo, hi)],
                    dtype=np.int8)
    NCLS = len(classes)
    dem_cls = np.zeros((NKEY, NCLS), np.int32)
    np.add.at(dem_cls, (gkey, ecls), 1)

    key_chunk = (np.arange(NKEY) // 128) % NCHUNK

    # interval-constraint DP for per-chunk widths (joint across cores):
    # c[k+1] = max_i (c[i] + dem[i,k]) where dem[i,k] = max over keys of
    # edges whose interval is within [i, k]
    W = np.zeros((NCHUNK, NW), np.int64)
    for j in range(NCHUNK):
        sel = key_chunk == j
        dj = dem_cls[sel]
        dem = {}
        for i in range(NW):
            for k in range(i, NW):
                csel = [cidx[c] for c in classes
                        if c[0] >= i and c[1] <= k]
                dem[(i, k)] = int(dj[:, csel].sum(axis=1).max()) if csel else 0
        c = [0] * (NW + 1)
        for k in range(NW):
            c[k + 1] = max([c[i] + dem[(i, k)] for i in range(k + 1)] + [c[k]])
        c[1] = max(c[1], 1)
        for k in range(1, NW):
            c[k + 1] = max(c[k + 1], c[k])
        W[j] = np.diff(np.array(c))
    assert (W[:, 0] >= 1).all()

    # per-key greedy assignment: for w in 0..NW-1 take classes by ascending hi
    remaining = dem_cls.astype(np.int64).copy()
    take = np.zeros((NKEY, NCLS, NW), np.int32)   # edges of class -> window
    order = sorted(range(NCLS), key=lambda ci: (classes[ci][1], classes[ci][0]))
    for w in range(NW):
        cap = W[key_chunk, w].copy()
        for ci in order:
            clo, chi = classes[ci]
            if not (clo <= w <= chi):
                continue
            if chi == w:
                t = remaining[:, ci].copy()   # must take all
            else:
                t = np.minimum(remaining[:, ci], cap)
            take[:, ci, w] = t
            cap -= t
            remaining[:, ci] -= t
        assert (cap >= 0).all(), f"window {w} overflow"
    assert (remaining == 0).all(), "assignment infeasible"

    # per-edge window: position within (key, class) decides the window
    eorder = np.lexsort((ecls, gkey))
    kc_sorted = gkey[eorder] * np.int64(NCLS) + ecls[eorder]
    starts = np.r_[0, np.flatnonzero(np.diff(kc_sorted)) + 1]
    counts = np.diff(np.r_[starts, EE])
    posin = np.arange(EE) - np.repeat(starts, counts)
    cum = np.cumsum(take, axis=2)               # [NKEY, NCLS, NW]
    pos_e = np.empty(EE, np.int64)
    pos_e[eorder] = posin
    cum_e = cum[gkey, ecls]                     # [EE, NW]
    win_e = (pos_e[:, None] >= cum_e).sum(axis=1).astype(np.int8)
    assert (win_e < NW).all()
    assert (cov[win_e, np.arange(EE)]).all()

    # depth within (key, window)
    kw = gkey * np.int64(NW) + win_e
    o2 = np.argsort(kw, kind="stable")
    kw_s = kw[o2]
    st2 = np.r_[0, np.flatnonzero(np.diff(kw_s)) + 1]
    cn2 = np.diff(np.r_[st2, EE])
    k_s = np.arange(EE) - np.repeat(st2, cn2)
    depth_e = np.empty(EE, np.int64)
    depth_e[o2] = k_s
    assert (depth_e < W[dst_chunk, win_e]).all()
    return W, win_e, depth_e


def _prepare(x, edge_index, edge_attr,
             W_res, b_res, alpha_mix,
             W1, att_src1, att_dst1, We1, att_e1, b1,
             W2, att_src2, att_dst2, We2, att_e2, b2):
    N, D = x.shape
    E = edge_index.shape[1]
    f32 = np.float32

    src = np.concatenate([edge_index[0], np.arange(N, dtype=np.int64)]).astype(np.int64)
    dst = np.concatenate([edge_index[1], np.arange(N, dtype=np.int64)]).astype(np.int64)
    ea = np.concatenate([edge_attr[:, 0].astype(f32),
                         np.full(N, edge_attr.astype(f32).mean(), dtype=f32)])
    EE = E + N

    deg = np.bincount(dst, minlength=N)

    PCORE = int(math.ceil(N / NCORE / 128) * 128)
    NCHUNK = PCORE // 128
    TROWS = NCORE * PCORE
    B0 = TROWS - LIM
    S = tuple(int(round(i * B0 / (NW - 1))) for i in range(NW))

    # Deal nodes to cores round-robin in degree order (balances edge counts);
    # within a core sort by degree so chunk members have similar degree.
    order = np.argsort(deg, kind="stable")
    rank = np.empty(N, dtype=np.int64)
    rank[order] = np.arange(N)
    core_of = (rank % NCORE).astype(np.int32)
    pos_in_core = np.full(N, -1, dtype=np.int64)
    perm = np.full((NCORE, PCORE), -1, dtype=np.int64)
    for c in range(NCORE):
        nodes = np.where(core_of == c)[0]
        nodes = nodes[np.argsort(deg[nodes], kind="stable")]
        pos_in_core[nodes] = np.arange(len(nodes))
        perm[c, :len(nodes)] = nodes
    p_id = core_of.astype(np.int64) * PCORE + pos_in_core

    src_p = p_id[src]
    dst_core = core_of[dst]
    dst_chunk = pos_in_core[dst] // 128
    dst_lane = pos_in_core[dst] % 128

    W, win_e, depth_e = _pack(src_p, dst_core, dst_chunk, dst_lane, NCHUNK, S)

    # Greedy group packing: chunks -> groups with <= B_MAX blocks each.
    # Group packing with a ramp: small first groups so the vector engine
    # starts while later gathers stream, and a small final group so the
    # exposed tail compute is short.
    wsum = W.sum(axis=1)
    groups = []
    gs, acc = 0, 0
    for j in range(NCHUNK):
        b = int(wsum[j])
        cap = (B_MAX // 3 if len(groups) == 0 else
               2 * B_MAX // 3 if len(groups) == 1 else B_MAX)
        if acc + b > cap and j > gs:
            groups.append((gs, j))
            gs, acc = j, 0
        acc += b
    groups.append((gs, NCHUNK))
    # Down-ramp the tail: the final group's gather + reduce + epilogue +
    # AllGather piece is the exposed critical path into edge phase 2, so
    # split trailing groups until the last one is small.
    while True:
        a, b = groups[-1]
        if b - a < 2 or wsum[a:b].sum() <= B_MAX // 3:
            break
        mid = a + (b - a) * 2 // 3
        if mid == a:
            break
        groups[-1:] = [(a, mid), (mid, b)]

    # Block layout per group: [win0 blocks of its chunks | win1 | win2 | win3]
    base = np.zeros((NCHUNK, NW), np.int64)
    group_info = []   # (blk0, (nblk per window), (ca, cb))
    bpos = 0
    for (a, b) in groups:
        blk0 = bpos
        nblk = []
        for w in range(NW):
            n0 = bpos
            for j in range(a, b):
                base[j, w] = bpos
                bpos += int(W[j, w])
            nblk.append(bpos - n0)
        group_info.append((blk0, tuple(nblk), (a, b)))
    B_TOT = bpos
    NSLOT = B_TOT * 128

    blk_e = base[dst_chunk, win_e] + depth_e
    slot_e = blk_e * 128 + dst_lane
    idx_val = (src_p - np.asarray(S, dtype=np.int64)[win_e])
    assert (idx_val >= 0).all() and (idx_val < LIM).all()
    idx_val = idx_val.astype(np.int16)

    c1 = float(np.dot(We1[0].astype(f32), att_e1.astype(f32)))
    c2 = float(np.dot(We2[0].astype(f32), att_e2.astype(f32)))

    idx_imgs, ae1_imgs, ae2_imgs = [], [], []
    for c in range(NCORE):
        m = dst_core == c
        sl = slot_e[m]
        grid_idx = np.zeros(NSLOT, dtype=np.int16)
        grid_idx[sl] = idx_val[m]
        g1 = np.full(NSLOT, PAD_AE, dtype=f32)
        g1[sl] = c1 * ea[m]
        g2 = np.full(NSLOT, PAD_AE, dtype=f32)
        g2[sl] = c2 * ea[m]
        img16 = grid_idx.reshape(-1, 16).T
        # dma_gather on queue 0 reads 32 idx channels; 2x-replicated 16-wrap
        idx_imgs.append(np.tile(img16, (2, 1)).copy())
        ae1_imgs.append(np.ascontiguousarray(g1.reshape(B_TOT, 128).T.astype(np.float16)))
        ae2_imgs.append(np.ascontiguousarray(g2.reshape(B_TOT, 128).T.astype(np.float16)))

    # Give fully-padded lanes (node-count padding) one live slot so s > 0.
    lane_has = np.zeros((NCORE, PCORE), dtype=bool)
    lane_has[dst_core, pos_in_core[dst]] = True
    for c in range(NCORE):
        for j in range(NCHUNK):
            dead = np.where(~lane_has[c, j * 128:(j + 1) * 128])[0]
            if len(dead):
                ae1_imgs[c][dead, base[j, 0]] = 0.0
                ae2_imgs[c][dead, base[j, 0]] = 0.0

    # Weights.  Residual Linear folded into layer 1; biases folded via the
    # ones row of the lhsT (layer 1) / zero-bias (layer 2).
    W_res = W_res.astype(f32)
    b_res = b_res.astype(f32)
    W1 = W1.astype(f32)
    W2 = W2.astype(f32)
    alpha = float(alpha_mix)
    W1e = W_res @ W1
    b1e = b_res @ W1

    # Table pass rhs, layer 1: cols [xl(0:64) | one | a_src], lhsT rows [x; 1]
    Wb1_tab = np.zeros((D + 1, TCOL), dtype=f32)
    Wb1_tab[:D, 0:D] = W1e
    Wb1_tab[D, 0:D] = b1e
    Wb1_tab[D, D] = 1.0
    Wb1_tab[:D, D + 1] = W1e @ att_src1.astype(f32)
    Wb1_tab[D, D + 1] = float(b1e @ att_src1.astype(f32))
    # Own pass rhs, layer 1: cols [a_dst | alpha*xres(0:64)]
    Wb1_own = np.zeros((D + 1, 1 + D), dtype=f32)
    Wb1_own[:D, 0] = W1e @ att_dst1.astype(f32)
    Wb1_own[D, 0] = float(b1e @ att_dst1.astype(f32))
    Wb1_own[:D, 1:] = W_res * alpha
    Wb1_own[D, 1:] = b_res * alpha
    # Table pass rhs, layer 2 (lhsT rows [h; 1])
    Wb2_tab = np.zeros((D + 1, TCOL), dtype=f32)
    Wb2_tab[:D, 0:D] = W2
    Wb2_tab[D, D] = 1.0
    Wb2_tab[:D, D + 1] = W2 @ att_src2.astype(f32)
    # Own pass rhs, layer 2: col [a_dst2]
    Wb2_own = np.zeros((D + 1, 1), dtype=f32)
    Wb2_own[:D, 0] = W2 @ att_dst2.astype(f32)

    # Dense inputs: x^T in p_id order with ones row, fp16.
    xT_full = np.zeros((D + 1, TROWS), dtype=np.float16)
    valid_all = perm.reshape(-1) >= 0
    cols = np.arange(TROWS)[valid_all]
    xT_full[:D, cols] = x[perm.reshape(-1)[valid_all]].astype(np.float16).T
    xT_full[D, :] = 1.0

    WMAXW = int(W.max())
    WMAXC = int(W.sum(axis=1).max())

    cfg = dict(
        N=N, D=D, PCORE=PCORE, NCHUNK=NCHUNK, TROWS=TROWS,
        B_TOT=B_TOT, NSLOT=NSLOT, WMAXW=WMAXW, WMAXC=WMAXC, S=S,
        W=tuple(tuple(int(v) for v in row) for row in W),
        base=tuple(tuple(int(v) for v in row) for row in base),
        groups=tuple((int(b0), tuple(int(n) for n in nblk), (int(a), int(b)))
                     for (b0, nblk, (a, b)) in group_info),
    )

    in_maps = []
    for c in range(NCORE):
        in_maps.append(dict(
            xT_full=xT_full,
            xT_own=np.ascontiguousarray(xT_full[:, c * PCORE:(c + 1) * PCORE]),
            idx_img=idx_imgs[c],
            ae1=ae1_imgs[c],
            ae2=ae2_imgs[c],
            Wb1_tab=Wb1_tab.astype(np.float16),
            Wb1_own=Wb1_own.astype(np.float16),
            Wb2_tab=Wb2_tab.astype(np.float16),
            Wb2_own=Wb2_own.astype(np.float16),
            b1row=np.tile(b1.astype(f32).reshape(1, D), (128, 1)),
            b2row=np.tile(b2.astype(f32).reshape(1, D), (128, 1)),
            ones_row=np.ones((1, PCORE), dtype=np.float16),
            ident=np.eye(128, dtype=np.float16),
        ))
    return cfg, in_maps, perm


# ----------------------------------------------------------------------------
# Device program
# ----------------------------------------------------------------------------

def _build(cfg_key, stage='full'):
    import contextlib

    import concourse.bass as bass
    import concourse.tile as tile
    import concourse.mybir as mybir
    from concourse import bacc
    from concourse.library_config import mlp

    cfg = dict(cfg_key)
    D = cfg["D"]
    PCORE, NCHUNK, TROWS = cfg["PCORE"], cfg["NCHUNK"], cfg["TROWS"]
    B_TOT, NSLOT, WMAXW = cfg["B_TOT"], cfg["NSLOT"], cfg["WMAXW"]
    WMAXC = cfg["WMAXC"]
    S = cfg["S"]
    W = cfg["W"]
    base = cfg["base"]
    groups = cfg["groups"]

    fp16 = mybir.dt.float16
    fp32 = mybir.dt.float32
    i16 = mybir.dt.int16
    AF = mybir.ActivationFunctionType
    ALU = mybir.AluOpType

    nc = bacc.Bacc("TRN2", target_bir_lowering=False, debug=False,
                   num_devices=NCORE, dynamic_dma_scratch_size=SCRATCH)

    xT_full_d = nc.dram_tensor("xT_full", [D + 1, TROWS], fp16, kind="ExternalInput")
    xT_own_d = nc.dram_tensor("xT_own", [D + 1, PCORE], fp16, kind="ExternalInput")
    idx_img = nc.dram_tensor("idx_img", [32, NSLOT // 16], i16, kind="ExternalInput")
    ae1_d = nc.dram_tensor("ae1", [128, B_TOT], fp16, kind="ExternalInput")
    ae2_d = nc.dram_tensor("ae2", [128, B_TOT], fp16, kind="ExternalInput")
    Wb1_tab_d = nc.dram_tensor("Wb1_tab", [D + 1, TCOL], fp16, kind="ExternalInput")
    Wb1_own_d = nc.dram_tensor("Wb1_own", [D + 1, 1 + D], fp16, kind="ExternalInput")
    Wb2_tab_d = nc.dram_tensor("Wb2_tab", [D + 1, TCOL], fp16, kind="ExternalInput")
    Wb2_own_d = nc.dram_tensor("Wb2_own", [D + 1, 1], fp16, kind="ExternalInput")
    b1row_d = nc.dram_tensor("b1row", [128, D], fp32, kind="ExternalInput")
    b2row_d = nc.dram_tensor("b2row", [128, D], fp32, kind="ExternalInput")
    ones_d = nc.dram_tensor("ones_row", [1, PCORE], fp16, kind="ExternalInput")
    ident_d = nc.dram_tensor("ident", [128, 128], fp16, kind="ExternalInput")
    y_d = nc.dram_tensor("y", [PCORE, D], fp32, kind="ExternalOutput")

    T_d = [nc.dram_tensor(f"T{l}", [TROWS, ROW], fp16) for l in range(2)]

    # Piece split of the group list: the h AllGather is pipelined in NQ
    # column pieces, each issued as soon as its chunks' epilogue is done so
    # the collective overlaps the remaining edge-phase groups.
    NGRP = len(groups)
    # ~2 groups per piece so the serial collective pipe starts early and
    # never stacks.  The tail pieces' h only exists at edge-1's end, so each
    # trailing piece pays its 15us launch overhead serially: merge the last
    # few (small, down-ramped) groups into one final piece.
    qbound = [b for b in range(2, NGRP - 2, 2)]
    qbound.append(NGRP)
    NQ = len(qbound)
    qruns = []
    g0 = 0
    for q in range(NQ):
        g1 = qbound[q]
        ca = groups[g0][2][0]
        cb = groups[g1 - 1][2][1]
        qruns.append((g1 - 1, ca, cb))
        g0 = g1
    # h travels fp8: per-node a_dst error cancels in the segment softmax
    # (constant shift per destination), and the xl2/a_src2 error is averaged
    # over D=64 terms.  Rows [h(64) | ones] so the table-2 matmul keeps its
    # denominator column without a separate ones load.
    fp8 = mybir.dt.float8e4
    h_tin_q = [nc.dram_tensor(f"h_tin{q}", [D + 1, (cb - ca) * 128], fp8)
               for q, (_, ca, cb) in enumerate(qruns)]
    h_tall_q = [nc.dram_tensor(f"h_tall{q}", [NCORE * (D + 1), (cb - ca) * 128],
                               fp8, addr_space="Shared")
                for q, (_, ca, cb) in enumerate(qruns)]

    nc.gpsimd.load_library(mlp)
    rg = [list(range(NCORE))]

    with tile.TileContext(nc) as tc:
        with contextlib.ExitStack() as ctx:
            resident = ctx.enter_context(tc.tile_pool(name="resident", bufs=1))
            slab = ctx.enter_context(tc.tile_pool(name="slab", bufs=2))
            gpool = ctx.enter_context(tc.tile_pool(name="gather", bufs=2))
            ppool = ctx.enter_context(tc.tile_pool(name="prod", bufs=2))
            spool = ctx.enter_context(tc.tile_pool(name="small", bufs=3))
            epool = ctx.enter_context(tc.tile_pool(name="epil", bufs=2))
            dpool = ctx.enter_context(tc.tile_pool(name="dense", bufs=3))
            ipool = ctx.enter_context(tc.tile_pool(name="idx", bufs=3))
            psum_p = ctx.enter_context(tc.tile_pool(name="ps", bufs=6, space="PSUM"))
            psum_t = ctx.enter_context(tc.tile_pool(name="pst", bufs=2, space="PSUM"))

            # ---------------- resident loads ----------------
            Wb1_tab_sb = resident.tile([D + 1, TCOL], fp16)
            nc.sync.dma_start(Wb1_tab_sb[:], Wb1_tab_d.ap())
            Wb1_own_sb = resident.tile([D + 1, 1 + D], fp16)
            nc.sync.dma_start(Wb1_own_sb[:], Wb1_own_d.ap())
            Wb2_tab_sb = resident.tile([D + 1, TCOL], fp16)
            nc.sync.dma_start(Wb2_tab_sb[:], Wb2_tab_d.ap())
            Wb2_own_sb = resident.tile([D + 1, 1], fp16)
            nc.sync.dma_start(Wb2_own_sb[:], Wb2_own_d.ap())
            b1row = resident.tile([128, D], fp32)
            nc.sync.dma_start(b1row[:], b1row_d.ap())
            b2row = resident.tile([128, D], fp32)
            nc.sync.dma_start(b2row[:], b2row_d.ap())
            ident = resident.tile([128, 128], fp16)
            nc.sync.dma_start(ident[:], ident_d.ap())
            expshift = resident.tile([128, 1], fp32)
            nc.vector.memset(expshift[:], EXP_SHIFT)

            h_T = resident.tile([D + 1, PCORE], fp8)
            nc.vector.memset(h_T[D:D + 1, :], 1.0)
            Wb2_tab8 = resident.tile([D + 1, TCOL], fp8)
            nc.vector.tensor_copy(Wb2_tab8[:], Wb2_tab_sb[:])
            Wb2_own8 = resident.tile([D + 1, 1], fp8)
            nc.vector.tensor_copy(Wb2_own8[:], Wb2_own_sb[:])

            ae_sb = [resident.tile([128, B_TOT], fp16, name=f"ae_sb{l}")
                     for l in range(2)]
            aeadst = [resident.tile([128, B_TOT], fp16, name=f"aeadst{l}")
                      for l in range(2)]
            xres16 = resident.tile([128, NCHUNK * D], fp16)
            h_sb = resident.tile([128, NCHUNK * D], fp16)
            adst = [resident.tile([128, NCHUNK], fp32, name=f"adst{l}")
                    for l in range(2)]
            pre_buf = resident.tile([128, NCHUNK * D], fp32)

            def table_pass(layer, lhsT_src):
                """Write the full gather table T[layer] from dense matmuls.
                lhsT_src(s) -> loads slab s ([D+1, PCORE]) and returns tile.
                One batched DMA write per slab (per-chunk writes serialize on
                the HWDGE fixed overhead); PSUM->SBUF staging alternates
                between the Activation and Vector engines."""
                half = (NCHUNK + 2) // 3
                for s in range(NCORE):
                    xs = lhsT_src(s)
                    for j0 in range(0, NCHUNK, half):
                        j1 = min(j0 + half, NCHUNK)
                        tb = dpool.tile([128, half, TCOL], fp16, tag="tabs")
                        for j in range(j0, j1):
                            ps = psum_p.tile([128, TCOL], fp32, tag="dps")
                            nc.tensor.matmul(ps[:], xs[:, j * 128:(j + 1) * 128],
                                             (Wb1_tab_sb if layer == 0 else Wb2_tab_sb)[:],
                                             start=True, stop=True)
                            if j % 2 == 0:
                                nc.scalar.activation(tb[:, j - j0, :], ps[:], AF.Copy)
                            else:
                                nc.vector.tensor_copy(tb[:, j - j0, :], ps[:])
                        r0 = (s * NCHUNK + j0) * 128
                        nc.sync.dma_start(
                            T_d[layer].ap()[r0:r0 + (j1 - j0) * 128, 0:TCOL]
                            .rearrange("(b l) c -> l b c", l=128),
                            tb[:, 0:j1 - j0, :])

            def own_pass(layer, ja=0, jb=NCHUNK, lhsT=None):
                if layer == 0:
                    xo = slab.tile([D + 1, PCORE], fp16, tag="slab")
                    nc.sync.dma_start(xo[:], xT_own_d.ap())
                    lhsT = xo
                elif lhsT is None:
                    lhsT = h_T
                ncols = (1 + D) if layer == 0 else 1
                W_own = Wb1_own_sb if layer == 0 else Wb2_own8
                for j in range(ja, jb):
                    ps = psum_p.tile([128, TCOL], fp32, tag="dps")
                    nc.tensor.matmul(ps[:, 0:ncols], lhsT[:, j * 128:(j + 1) * 128],
                                     W_own[:], start=True, stop=True)
                    nc.vector.tensor_copy(adst[layer][:, j:j + 1], ps[:, 0:1])
                    if layer == 0:
                        nc.scalar.activation(
                            xres16[:, j * D:(j + 1) * D], ps[:, 1:1 + D], AF.Copy)

            def quarter_epilogue(q):
                """h = elu(pre + b1) for quarter q's chunks, transpose into
                h_T, write h_tin[q] and kick its AllGather.  Emitted mid
                edge-phase-1 so the collective overlaps later groups."""
                _, ca, cb = qruns[q]
                for j0 in range(ca, cb, 4):
                    j1 = min(j0 + 4, cb)
                    b0, b1_ = j0 * D, j1 * D
                    w = b1_ - b0
                    nj = j1 - j0
                    t0 = epool.tile([128, 4 * D], fp32, tag="eb0")
                    nc.vector.tensor_tensor(
                        t0[:, 0:w].rearrange("l (j c) -> l j c", c=D),
                        pre_buf[:, b0:b1_].rearrange("l (j c) -> l j c", c=D),
                        b1row[:].unsqueeze(1).broadcast_to([128, nj, D]), ALU.add)
                    mneg = epool.tile([128, 4 * D], fp32, tag="eb1")
                    nc.vector.tensor_scalar_min(mneg[:, 0:w], t0[:, 0:w], 0.0)
                    eneg = epool.tile([128, 4 * D], fp32, tag="eb2")
                    nc.scalar.activation(eneg[:, 0:w], mneg[:, 0:w], AF.Exp)
                    ppos = epool.tile([128, 4 * D], fp32, tag="eb1b")
                    nc.vector.tensor_scalar_max(ppos[:, 0:w], t0[:, 0:w], 0.0)
                    nc.vector.scalar_tensor_tensor(
                        h_sb[:, b0:b1_], eneg[:, 0:w], -1.0, ppos[:, 0:w],
                        ALU.add, ALU.add)
                for j in range(ca, cb):
                    pt = psum_t.tile([D, 128], fp16, tag="pt")
                    nc.tensor.transpose(pt[:], h_sb[:, j * D:(j + 1) * D], ident[:])
                    nc.vector.tensor_copy(h_T[0:D, j * 128:(j + 1) * 128], pt[:])

            def launch_collective(q):
                """h_tin write + AllGather for piece q.  Emitted one group
                after the epilogue compute so its sem waits don't stall the
                in-order SP/Pool queues mid-stream."""
                _, ca, cb = qruns[q]
                nc.sync.dma_start(h_tin_q[q].ap(),
                                  h_T[:, ca * 128:cb * 128])
                nc.gpsimd.collective_compute(
                    "AllGather", ALU.bypass, replica_groups=rg,
                    ins=[h_tin_q[q].ap().opt()], outs=[h_tall_q[q].ap().opt()])

            def load_ae(layer):
                nc.sync.dma_start(ae_sb[layer][:],
                                  (ae1_d if layer == 0 else ae2_d).ap())

            def prep_aeadst(layer, ja=0, jb=NCHUNK):
                """ae + a_dst per slot for chunks [ja, jb)."""
                for j in range(ja, jb):
                    for w in range(NW):
                        if W[j][w]:
                            b0 = base[j][w]
                            nc.vector.tensor_scalar_add(
                                aeadst[layer][:, b0:b0 + W[j][w]],
                                ae_sb[layer][:, b0:b0 + W[j][w]],
                                adst[layer][:, j:j + 1])

            def edge_phase(layer, tasks=None):
                table = T_d[layer]
                for gi, (blk0, nblk, (ca, cb)) in enumerate(groups):
                    bg = sum(nblk)
                    G = gpool.tile([128, B_MAX, ROW], fp16, tag="G")
                    it = ipool.tile([32, B_MAX * 8], i16, tag="it")
                    nc.sync.dma_start(it[:, 0:bg * 8],
                                      idx_img.ap()[:, blk0 * 8:(blk0 + bg) * 8])
                    off = 0
                    for w in range(NW):
                        for s0 in range(0, nblk[w], SUB_BLK):
                            nb = min(SUB_BLK, nblk[w] - s0)
                            o = off + s0
                            nc.gpsimd.dma_gather(
                                G[:, o:o + nb, :],
                                table.ap()[S[w]:S[w] + LIM, :],
                                it[:, o * 8:(o + nb) * 8], nb * 128, nb * 128, ROW)
                        off += nblk[w]
                    u = spool.tile([128, B_MAX], fp32, tag="u")
                    nc.vector.tensor_tensor(
                        u[:, 0:bg], G[:, 0:bg, D + 1:D + 2].squeeze(2),
                        aeadst[layer][:, blk0:blk0 + bg], ALU.add)
                    t = spool.tile([128, B_MAX], fp32, tag="t")
                    nc.vector.scalar_tensor_tensor(
                        t[:, 0:bg], u[:, 0:bg], 0.2, u[:, 0:bg],
                        ALU.mult, ALU.max)
                    ex = spool.tile([128, B_MAX], fp16, tag="ex")
                    nc.scalar.activation(ex[:, 0:bg], t[:, 0:bg], AF.Exp,
                                         bias=expshift[:])
                    for j in range(ca, cb):
                        # P holds the chunk's windows back to back so one
                        # reduce covers the whole neighborhood.
                        P = ppool.tile([128, WMAXC, D + 1], fp16, tag="P")
                        po = 0
                        for w in range(NW):
                            dd = W[j][w]
                            if not dd:
                                continue
                            r0 = base[j][w] - blk0
                            nc.vector.tensor_tensor(
                                P[:, po:po + dd, :], G[:, r0:r0 + dd, 0:D + 1],
                                ex[:, r0:r0 + dd].unsqueeze(2)
                                .broadcast_to([128, dd, D + 1]),
                                ALU.mult)
                            po += dd
                        acc = spool.tile([128, D + 1], fp32, tag="red")
                        nc.vector.tensor_reduce(
                            acc[:], P[:, 0:po, :].transpose([0, 2, 1]),
                            axis=mybir.AxisListType.X, op=ALU.add)
                        rs = spool.tile([128, 1], fp32, tag="rs")
                        nc.vector.reciprocal(rs[:], acc[:, D:D + 1])
                        nc.vector.tensor_scalar_mul(
                            pre_buf[:, j * D:(j + 1) * D], acc[:, 0:D], rs[:])
                    if tasks:
                        for fn in tasks.get(gi, ()):
                            fn()

            def y_quarter(q):
                """y = pre + b2 + alpha*x_res for quarter q's chunks, written
                out as soon as they are reduced (overlaps later edge-2 groups)."""
                _, ca, cb = qruns[q]
                for j0 in range(ca, cb, 4):
                    j1 = min(j0 + 4, cb)
                    b0, b1_ = j0 * D, j1 * D
                    w = b1_ - b0
                    nj = j1 - j0
                    y0 = epool.tile([128, 4 * D], fp32, tag="eb0")
                    nc.vector.tensor_tensor(
                        y0[:, 0:w].rearrange("l (j c) -> l j c", c=D),
                        pre_buf[:, b0:b1_].rearrange("l (j c) -> l j c", c=D),
                        b2row[:].unsqueeze(1).broadcast_to([128, nj, D]),
                        ALU.add)
                    y1 = epool.tile([128, 4 * D], fp32, tag="eb1")
                    nc.vector.tensor_tensor(y1[:, 0:w], y0[:, 0:w],
                                            xres16[:, b0:b1_], ALU.add)
                    nc.sync.dma_start(
                        y_d.ap().rearrange("(j l) c -> l j c", l=128)
                        [:, j0:j1, :],
                        y1[:, 0:w].rearrange("l (j c) -> l j c", c=D))

            def finish_early():
                y_stub = spool.tile([128, D], fp32, tag="ystub")
                nc.vector.memset(y_stub[:], 0.0)
                nc.sync.dma_start(y_d.ap()[0:128, :], y_stub[:])

            WFREE = NCHUNK * D
            NB = 512

            # ================= layer 1 =================
            def x_slab(s):
                xs = slab.tile([D + 1, PCORE], fp16, tag="slab")
                nc.sync.dma_start(xs[:], xT_full_d.ap()[:, s * PCORE:(s + 1) * PCORE])
                return xs

            QMAX = max(cb - ca for (_, ca, cb) in qruns)

            def table2_build(q, s0, s1):
                """Table-2 rows for piece q, source slabs [s0, s1).  Paced two
                pieces behind the AllGather launches so h_tall[q] is ready."""
                _, qa, qb = qruns[q]
                qw = (qb - qa) * 128
                qn = qb - qa
                for s in range(s0, s1):
                    xs = slab.tile([D + 1, QMAX * 128], fp8, tag="slab2")
                    nc.sync.dma_start(xs[:, 0:qw],
                                      h_tall_q[q].ap()[s * (D + 1):(s + 1) * (D + 1), :])
                    tb = dpool.tile([128, max(QMAX, (NCHUNK + 2) // 3), TCOL],
                                    fp16, tag="tabs")
                    for j in range(qn):
                        ps = psum_p.tile([128, TCOL], fp32, tag="dps")
                        nc.tensor.matmul(ps[:], xs[:, j * 128:(j + 1) * 128],
                                         Wb2_tab8[:], start=True, stop=True)
                        if j % 2 == 0:
                            nc.scalar.activation(tb[:, j, :], ps[:], AF.Copy)
                        else:
                            nc.vector.tensor_copy(tb[:, j, :], ps[:])
                    r0 = (s * NCHUNK + qa) * 128
                    nc.sync.dma_start(
                        T_d[1].ap()[r0:r0 + qn * 128, 0:TCOL]
                        .rearrange("(b l) c -> l b c", l=128),
                        tb[:, 0:qn, :])

            import functools
            table_pass(0, x_slab)
            own_pass(0)
            load_ae(0)
            load_ae(1)
            # aeadst prep is ~25us of small DVE ops; emitted all upfront it
            # delays the first groups' score compute (and thus G-tile reuse,
            # stalling the gather stream).  Prep just the first groups now,
            # the rest two groups ahead of use.
            prep_aeadst(0, groups[0][2][0], groups[min(2, NGRP - 1)][2][1])
            done = stage == "dense1"
            if not done:
                # Layer-2 prep is interleaved into edge phase 1 as post-group
                # tasks: epilogue (h + own2/prep2) at each piece's end group,
                # collective launch one group later, table-2 builds two pieces
                # behind (split into slab halves across adjacent groups).
                tasks0 = {}
                post0 = []

                def _at(gi, fn):
                    if gi < NGRP:
                        tasks0.setdefault(gi, []).append(fn)
                    else:
                        post0.append(fn)

                P = functools.partial
                # rest of layer-1 aeadst prep, two groups ahead of use
                for g in range(3, NGRP):
                    _at(g - 2, P(prep_aeadst, 0,
                                 groups[g][2][0], groups[g][2][1]))
                if stage != "edge1":
                    for q, (gi, ca, cb) in enumerate(qruns):
                        _at(gi, P(quarter_epilogue, q))
                        # Defer the launch one group so its sem wait doesn't
                        # stall the Pool queue mid-stream — except the last
                        # pieces, where collective earliness wins.
                        _at(gi if q >= NQ - 3 else gi + 1,
                            P(launch_collective, q))
                        # Layer-2 dense prep (own + aeadst) two groups after
                        # the epilogue that produced this piece's h columns:
                        # DVE has mid-stream slack, the trough doesn't.
                        _at(min(gi + 2, NGRP - 1), P(own_pass, 1, ca, cb))
                        _at(min(gi + 2, NGRP - 1), P(prep_aeadst, 1, ca, cb))
                    # Table-2 builds go after edge phase 1 (its DMA pipe is
                    # saturated; the trough's is free).  Slab-phased order:
                    # rows < 37632 (slabs 0-5) are written first across all
                    # pieces, which unblocks edge-2's window-0/1 gathers
                    # (range-based DRAM deps) while slabs 6-7 still build.
                    for q in range(NQ):
                        post0.append(P(table2_build, q, 0, 6))
                    for q in range(NQ):
                        post0.append(P(table2_build, q, 6, 7))
                    for q in range(NQ):
                        post0.append(P(table2_build, q, 7, NCORE))
                edge_phase(0, tasks0)
                for fn in post0:
                    fn()
                done = stage == "edge1"
            if done:
                finish_early()
            else:
                # ================= layer 2 =================
                if stage == "dense2":
                    finish_early()
                else:
                    tasks1 = {}
                    post1 = []
                    for q, (gi, ca, cb) in enumerate(qruns):
                        if gi + 1 < NGRP:
                            tasks1.setdefault(gi + 1, []).append(
                                functools.partial(y_quarter, q))
                        else:
                            post1.append(functools.partial(y_quarter, q))
                    edge_phase(1, tasks1)
                    for fn in post1:
                        fn()

    nc.compile()
    return nc


def _get_nc(cfg):
    import os
    stage = os.environ.get("KERNEL_STAGE", "full")
    key = (tuple(sorted(cfg.items())), stage)
    if key not in _BUILD_CACHE:
        _BUILD_CACHE[key] = _build(key[0], stage)
    return _BUILD_CACHE[key]


# ----------------------------------------------------------------------------
# Entry point
# ----------------------------------------------------------------------------

def kernel(**inputs):
    import sys
    if "/opt/trn_rl_repo" not in sys.path:
        sys.path.insert(0, "/opt/trn_rl_repo")
    from concourse.bass_utils import run_bass_kernel_spmd

    cfg, in_maps, perm = _prepare(**inputs)
    nc = _get_nc(cfg)
    res = run_bass_kernel_spmd(nc, in_maps, core_ids=list(range(NCORE)))
    kernel.last_results = res

    N, D = cfg["N"], cfg["D"]
    y = np.empty((N, D), dtype=np.float32)
    for c in range(NCORE):
        n = perm[c]
        valid = n >= 0
        y[n[valid]] = res.results[c]["y"][:valid.sum()]
    return y

